# revision 21
# baseline (speedup 1.0000x reference)
"""Trainium2 Bass kernel for nn_Atomic_node_only_lstm (GNN message passing + BiLSTM + MLP).

v2: s-paired GRU with msg-linear folded into the gate matmuls.

Data-parallel over 8 NeuronCores (batch N=65536 -> 8192/core), NB=512 cols/tile.

Device layout (per batch tile of NB columns), seq steps grouped in pairs
(groups: (s0,s1), (s2,s3), (s4,zero)):
  H_g  [128, NB] bf16: rows 0:48 h_a, 48:96 h_b, 96:112 att_a, 112:128 att_b
       (h rows indexed k*12+d, k = node; att rows indexed n*4+w)
  arb_n [96, NB]: att[n,w] replicated over d, slot a rows 0:48, slot b 48:96
  Q_n  [96, NB] = arb_n * H_g[0:96]   (q_n[w*12+d] = att[n,w]*h[w,d], both slots)
  hbar [96, NB]: rows j*48+n*12+d = sum_w q over w   (slot j, node n)
  GRU gates from K=128 contraction of H_g (h part + att part folds s_n*(W_ih@msg_b))
  plus K=96 contraction of hbar (folds (W_ih@msg_w)).
  PSUM super-tiles [96, 1024] span 2 banks: rz = [r | z], nh = [inn | hn].

LSTM: XL [96, NB] = [x_fw | x_bw], HL [96, NB] = [h_fw | h_bw], C [96, NB].
  Per gate-type bank halves in free dim: B_if = [i | f], B_go = [g | o].
  x[j] = h[(k,d)], j = d*4+k (handled in lhsT construction).
MLP reads HL directly (en1_w cols 0:48 fw, 48:96 bw).
"""
import numpy as np
import ml_dtypes
from contextlib import ExitStack

N, S, K, D, H = 65536, 5, 4, 12, 48
NCORES = 8
NCORE = N // NCORES          # 8192 batch elements per core
NB = 512                     # batch columns per tile
NTILES = NCORE // NB
NGROUPS = 3                  # s-pairs: (0,1), (2,3), (4,-)

BF = ml_dtypes.bfloat16


# ----------------------------------------------------------------------------
# host-side weight construction (numpy, all tiny)
# ----------------------------------------------------------------------------
def build_weights(msg_w, msg_b, gru_w_ih, gru_w_hh, gru_b_ih, gru_b_hh,
                  lstm_w_ih_fw, lstm_w_hh_fw, lstm_b_ih_fw, lstm_b_hh_fw,
                  lstm_w_ih_bw, lstm_w_hh_bw, lstm_b_ih_bw, lstm_b_hh_bw,
                  en1_w, en1_b, en2_w, en2_b, en3_w, en3_b):
    out = {}
    # composed message->gate matrices [12 out-gate-d, 12 in-feat]
    A_r = gru_w_ih[0:12] @ msg_w      # gi_r = A_r @ hbar + s*(w_ih_r@msg_b)
    A_z = gru_w_ih[12:24] @ msg_w
    A_n = gru_w_ih[24:36] @ msg_w
    bi_r = gru_w_ih[0:12] @ msg_b     # [12]
    bi_z = gru_w_ih[12:24] @ msg_b
    bi_n = gru_w_ih[24:36] @ msg_b

    # RA_n lhsT [32, 96] (placed at partitions 96:128 on device; rhs = H_g[96:128]).
    # rows: 0:16 att_a (n*4+w), 16:32 att_b. cols: j*48 + w*12 + d.
    for n in range(4):
        R = np.zeros((32, 96), np.float32)
        for j in range(2):
            for w in range(4):
                for d in range(12):
                    R[j * 16 + n * 4 + w, j * 48 + w * 12 + d] = 1.0
        out[f"RA{n}"] = R

    # SR_n lhsT [96, 96]: contract Q_n -> hbar contribution of node n.
    # Q_n rows j*48 + w*12 + d ; hbar cols j*48 + n*12 + d.
    for n in range(4):
        Sm = np.zeros((96, 96), np.float32)
        for j in range(2):
            for w in range(4):
                for d in range(12):
                    Sm[j * 48 + w * 12 + d, j * 48 + n * 12 + d] = 1.0
        out[f"SR{n}"] = Sm

    # GRU gate lhsTs.
    # H-part [128, 96]: rhs = H_g (h rows j*48+k*12+dp, att rows 96+j*16+k*4+w)
    # out cols j*48 + k*12 + d.
    def gate_H(whh_blk, bi_blk):
        Wm = np.zeros((128, 96), np.float32)
        for j in range(2):
            for k in range(4):
                for d in range(12):
                    c = j * 48 + k * 12 + d
                    if whh_blk is not None:
                        for dp in range(12):
                            Wm[j * 48 + k * 12 + dp, c] = whh_blk[d, dp]
                    if bi_blk is not None:
                        for w in range(4):
                            Wm[96 + j * 16 + k * 4 + w, c] = bi_blk[d]
        return Wm

    def gate_HB(A_blk):
        Wm = np.zeros((96, 96), np.float32)
        for j in range(2):
            for k in range(4):
                for d in range(12):
                    c = j * 48 + k * 12 + d
                    for dp in range(12):
                        Wm[j * 48 + k * 12 + dp, c] = A_blk[d, dp]
        return Wm

    out["W_r_H"] = gate_H(gru_w_hh[0:12], bi_r)
    out["W_z_H"] = gate_H(gru_w_hh[12:24], bi_z)
    out["W_hn_H"] = gate_H(gru_w_hh[24:36], None)[0:96]   # h rows only, K=96
    out["W_inn_att"] = gate_H(None, bi_n)[96:128]         # att rows only, K=32
    out["W_r_HB"] = gate_HB(A_r)
    out["W_z_HB"] = gate_HB(A_z)
    out["W_inn_HB"] = gate_HB(A_n)

    out["b_r96"] = np.tile(gru_b_ih[0:12] + gru_b_hh[0:12], 8).reshape(96, 1).astype(np.float32)
    out["b_z96"] = np.tile(gru_b_ih[12:24] + gru_b_hh[12:24], 8).reshape(96, 1).astype(np.float32)
    out["b_inn96"] = np.tile(gru_b_ih[24:36], 8).reshape(96, 1).astype(np.float32)
    out["b_hn96"] = np.tile(gru_b_hh[24:36], 8).reshape(96, 1).astype(np.float32)
    # rz super-bias [96,1] applies to both halves? No - bias is per partition, free
    # dim halves share it. r and z biases differ -> separate ACT calls? No:
    # sigma(rz super-tile) is ONE act over [96, 1024]; bias per-partition only.
    # So we must fold r/z biases into the matmuls instead: add via att-part?
    # Simpler: bias rows are per-partition; r occupies cols 0:512 and z 512:1024 of
    # the SAME partitions -> per-partition bias cannot differ. Fold biases into
    # lhsT via the constant-1 trick: use att rows? att sums vary. Instead keep
    # two ACT calls when biases differ. For rz we instead ADD the bias inside the
    # H-part lhsT using a constant row... no constant row exists in H_g.
    # Resolution: biases b_r/b_z are added via activation bias -> need separate
    # sigma(r), sigma(z). To keep ONE act per super-tile we instead pre-add the
    # bias by augmenting att columns: s_n terms already use att rows; a constant
    # offset cannot come from data. So: two ACTs for rz after all (see kernel).

    # LSTM chunk lhsTs [112, 112]: rhs = XH half = [x(0:48) | ones-row(48) pad |
    # h(64:112)]; row 48 carries the bias (XH[48]=1, rows 49:64 zero).
    # c1 cols = [i(0:48) | gap | o(64:112)], c2 cols = [g(0:48) | gap | f(64:112)].
    chunks = {"c1": ("I", "O"), "c2": ("G", "F")}
    gidx = {"I": 0, "F": 1, "G": 2, "O": 3}
    wih = {"fw": lstm_w_ih_fw, "bw": lstm_w_ih_bw}
    whh = {"fw": lstm_w_hh_fw, "bw": lstm_w_hh_bw}
    bih = {"fw": lstm_b_ih_fw, "bw": lstm_b_ih_bw}
    bhh = {"fw": lstm_b_hh_fw, "bw": lstm_b_hh_bw}
    for cname, (ga, gb) in chunks.items():
        for dirn in ("fw", "bw"):
            M = np.zeros((112, 112), np.float32)
            for col0, gname in ((0, ga), (64, gb)):
                g = gidx[gname]
                wg = wih[dirn][g * 48:(g + 1) * 48, :]   # [48 out m, 48 xin jj]
                hg = whh[dirn][g * 48:(g + 1) * 48, :]
                for k in range(4):
                    for d in range(12):
                        M[k * 12 + d, col0:col0 + 48] = wg[:, d * 4 + k]
                M[48, col0:col0 + 48] = (bih[dirn][g * 48:(g + 1) * 48]
                                         + bhh[dirn][g * 48:(g + 1) * 48])
                M[64:112, col0:col0 + 48] = hg.T
            out[f"L_{cname}_{dirn}"] = M

    # MLP: HN = [h_fw(0:48) | gap | h_bw(64:112)]
    We1 = np.zeros((112, 48), np.float32)
    We1[0:48, :] = en1_w[:, 0:48].T
    We1[64:112, :] = en1_w[:, 48:96].T
    out["We1"] = We1
    out["be1"] = en1_b.reshape(48, 1).astype(np.float32)
    out["ones16"] = np.ones((16, 2 * NB), np.float32)
    out["We2"] = en2_w.T.copy()                 # [48, 36]
    out["be2"] = en2_b.reshape(36, 1).astype(np.float32)
    out["We3"] = en3_w.T.copy()                 # [36, 6]
    out["be3"] = en3_b.reshape(6, 1).astype(np.float32)
    return out


def prep_inputs(nodes_feature, pos, attmat):
    """Full-batch host layout: feat [S, 48, N] bf16 (k-major (k,d)), att [S, 16, N] bf16."""
    nf = np.concatenate([nodes_feature, pos], axis=-1)       # [N,S,K,12]
    feat = np.ascontiguousarray(nf.transpose(1, 2, 3, 0)).reshape(S, 48, N)
    att = np.ascontiguousarray(attmat.transpose(1, 2, 3, 0)).reshape(S, 16, N)
    return feat.astype(BF), att.astype(BF)


# ----------------------------------------------------------------------------
# device kernel builder
# ----------------------------------------------------------------------------
def split_excess_waits(nc, max_waits=1):
    import concourse.mybir as mybir
    cnt = 0
    for f in nc.m.functions:
        for bb in f.blocks:
            insts = bb.instructions
            new = []
            changed = False
            for inst in insts:
                si = inst.sync_info
                waits = list(si.on_wait) if si and si.on_wait else []
                if len(waits) > max_waits:
                    changed = True
                    k = 0
                    while len(waits) - k > max_waits:
                        chunk = waits[k:k + max_waits]
                        k += max_waits
                        cnt += 1
                        nop = mybir.InstNoOp(name=f"waitsplit-{cnt}", ins=[], outs=[])
                        nop.engine = inst.engine
                        nop.sync_info = mybir.SyncInfo(on_wait=chunk, on_update=[])
                        new.append(nop)
                    inst.sync_info = mybir.SyncInfo(
                        on_wait=waits[k:],
                        on_update=list(si.on_update) if si.on_update else [])
                new.append(inst)
            if changed:
                bb.instructions = new
    return cnt


WEIGHT_SPECS = None  # filled in build_nc


def build_nc():
    import concourse.bass as bass
    import concourse.tile as tile
    from concourse import mybir

    f32 = mybir.dt.float32
    bf16 = mybir.dt.bfloat16
    AF = mybir.ActivationFunctionType
    ALU = mybir.AluOpType

    nc = bass.Bass("TRN2")

    feat_d = nc.dram_tensor("feat", [S, 48, NCORE], bf16, kind="ExternalInput")
    att_d = nc.dram_tensor("att", [S, 16, NCORE], bf16, kind="ExternalInput")

    wspecs = []
    for n in range(4):
        wspecs.append((f"RA{n}", (32, 96), bf16))
        wspecs.append((f"SR{n}", (96, 96), bf16))
    wspecs += [("W_r_H", (128, 96), bf16), ("W_z_H", (128, 96), bf16),
               ("W_hn_H", (96, 96), bf16), ("W_inn_att", (32, 96), bf16),
               ("W_r_HB", (96, 96), bf16), ("W_z_HB", (96, 96), bf16),
               ("W_inn_HB", (96, 96), bf16),
               ("b_r96", (96, 1), f32), ("b_z96", (96, 1), f32),
               ("b_inn96", (96, 1), f32), ("b_hn96", (96, 1), f32)]
    for cname in ("c1", "c2"):
        for dirn in ("fw", "bw"):
            wspecs.append((f"L_{cname}_{dirn}", (112, 112), bf16))
    wspecs.append(("ones16", (16, 2 * NB), bf16))
    wspecs += [("We1", (112, 48), bf16), ("be1", (48, 1), f32),
               ("We2", (48, 36), bf16), ("be2", (36, 1), f32),
               ("We3", (36, 6), bf16), ("be3", (6, 1), f32)]

    wnames = {}
    for nm, shp, dt in wspecs:
        wnames[nm] = nc.dram_tensor(nm, list(shp), dt, kind="ExternalInput")
    out_d = nc.dram_tensor("out", [6, NCORE], f32, kind="ExternalOutput")
    RA_QUAD = {0: 0, 1: 32, 2: 64, 3: 96}  # row-quadrant per RA matmul

    global WEIGHT_SPECS
    WEIGHT_SPECS = wspecs

    with tile.TileContext(nc) as tc:
        with ExitStack() as ctx:
            wpool = ctx.enter_context(tc.tile_pool(name="weights", bufs=1))
            wt = {}
            for nm, shp, dt in wspecs:
                # lhsT base partition must match the rhs row quadrant:
                # RA{n} runs at row quadrant 32n; W_inn_att at quadrant 96.
                if nm.startswith("RA"):
                    q = RA_QUAD[int(nm[2])]
                    t = wpool.tile([128, shp[1]], dt, tag=f"w_{nm}")
                    nc.sync.dma_start(t[q:q + 32, :], wnames[nm][:])
                    wt[nm] = t
                elif nm == "W_inn_att":
                    t = wpool.tile([128, shp[1]], dt, tag=f"w_{nm}")
                    nc.sync.dma_start(t[96:128, :], wnames[nm][:])
                    wt[nm] = t
                else:
                    t = wpool.tile([shp[0], shp[1]], dt, tag=f"w_{nm}")
                    nc.sync.dma_start(t[:], wnames[nm][:])
                    wt[nm] = t

            hpool = ctx.enter_context(tc.tile_pool(name="hg", bufs=2))
            arpool = ctx.enter_context(tc.tile_pool(name="ar", bufs=2))
            sbp = ctx.enter_context(tc.tile_pool(name="work", bufs=3))
            lstmp = ctx.enter_context(tc.tile_pool(name="lstm", bufs=2))
            psp = ctx.enter_context(tc.tile_pool(name="ps", bufs=2, space="PSUM"))

            for it in range(NTILES):
                c0 = it * NB
                # ---- load groups ----
                HG = []
                for g in range(NGROUPS):
                    hg = hpool.tile([128, NB], bf16, tag=f"HG{g}")
                    sa = 2 * g
                    if g == 2:
                        # zero slot-b rows first (partition offsets must be 32-aligned;
                        # DMAs below restore the overlapping real rows)
                        nc.vector.memset(hg[32:64, :], 0.0)
                        nc.vector.memset(hg[64:96, :], 0.0)
                        nc.vector.memset(hg[96:128, :], 0.0)
                    nc.sync.dma_start(hg[0:48, :], feat_d[sa, :, c0:c0 + NB])
                    nc.sync.dma_start(hg[96:112, :], att_d[sa, :, c0:c0 + NB])
                    if g < 2:
                        sb = 2 * g + 1
                        nc.sync.dma_start(hg[48:96, :], feat_d[sb, :, c0:c0 + NB])
                        nc.sync.dma_start(hg[112:128, :], att_d[sb, :, c0:c0 + NB])
                    HG.append(hg)

                # att replicated into all 4 row-quadrants so the 4 RA matmuls can
                # run concurrently in distinct PE row groups (quadrant 3 = hg).
                AT4 = []
                for g in range(NGROUPS):
                    at4 = hpool.tile([96, NB], bf16, tag=f"AT4{g}")
                    if g == 2:
                        nc.vector.memset(at4[:], 0.0)
                    for q in range(3):
                        nc.sync.dma_start(at4[32 * q:32 * q + 16, :],
                                          att_d[2 * g, :, c0:c0 + NB])
                        if g < 2:
                            nc.sync.dma_start(at4[32 * q + 16:32 * q + 32, :],
                                              att_d[2 * g + 1, :, c0:c0 + NB])
                    AT4.append(at4)

                # ---- attention replication for all groups ----
                ARBG = {}
                for g in range(NGROUPS):
                    hg = HG[g]
                    ps_a = psp.tile([96, 2 * NB], f32, tag="ps_rz", name=f"psa_{it}_{g}")
                    ps_b = psp.tile([96, 2 * NB], f32, tag="ps_nh", name=f"psb_{it}_{g}")
                    for n in range(4):
                        ps = (ps_a, ps_b)[n // 2]
                        sl = ps[:, (n % 2) * NB:(n % 2 + 1) * NB]
                        q = RA_QUAD[n]
                        rhs = (hg[96:128, :] if n == 3
                               else AT4[g][q:q + 32, :])
                        nc.tensor.matmul(sl, wt[f"RA{n}"][q:q + 32, :], rhs,
                                         start=True, stop=True,
                                         tile_position=(q, 0))
                        ar = arpool.tile([96, NB], bf16, tag=f"ARB{g}_{n}")
                        if n % 2 == 0:
                            nc.vector.tensor_copy(ar[:], sl)
                        else:
                            nc.scalar.copy(ar[:], sl)
                        ARBG[(g, n)] = ar

                # ---- 2 GRU passes, groups interleaved (3 independent chains) ----
                for pas in range(2):
                    for g in range(NGROUPS):
                        hg = HG[g]
                        ARB = [ARBG[(g, n)] for n in range(4)]
                        # Q_n = arb_n * h (both slots at once)
                        Q = []
                        for n in range(4):
                            q = sbp.tile([96, NB], bf16, tag=f"Q{n}")
                            if n >= 2:
                                nc.gpsimd.tensor_tensor(q[:], ARB[n][:], hg[0:96, :],
                                                        ALU.mult)
                            else:
                                nc.vector.tensor_mul(q[:], ARB[n][:], hg[0:96, :])
                            Q.append(q)
                        # hbar = sum_n SR_n @ Q_n
                        ps_hb = psp.tile([96, 2 * NB], f32, tag="ps_nh",
                                         name=f"pshb_{it}_{g}_{pas}")
                        for n in range(4):
                            nc.tensor.matmul(ps_hb[:, 0:NB], wt[f"SR{n}"][:], Q[n][:],
                                             start=(n == 0), stop=(n == 3))
                        hb = sbp.tile([96, NB], bf16, tag="HBs")
                        nc.vector.tensor_copy(hb[:], ps_hb[:, 0:NB])

                        # gates
                        ps_rz = psp.tile([96, 2 * NB], f32, tag="ps_rz",
                                         name=f"psrz_{it}_{g}_{pas}")
                        nc.tensor.matmul(ps_rz[:, 0:NB], wt["W_r_H"][:], hg[0:128, :],
                                         start=True, stop=False)
                        nc.tensor.matmul(ps_rz[:, 0:NB], wt["W_r_HB"][:], hb[:],
                                         start=False, stop=True)
                        nc.tensor.matmul(ps_rz[:, NB:2 * NB], wt["W_z_H"][:], hg[0:128, :],
                                         start=True, stop=False)
                        nc.tensor.matmul(ps_rz[:, NB:2 * NB], wt["W_z_HB"][:], hb[:],
                                         start=False, stop=True)
                        ps_nh = psp.tile([96, 2 * NB], f32, tag="ps_nh",
                                         name=f"psnh_{it}_{g}_{pas}")
                        nc.tensor.matmul(ps_nh[:, 0:NB], wt["W_inn_att"][96:128, :],
                                         hg[96:128, :], start=True, stop=False,
                                         tile_position=(96, 0))
                        nc.tensor.matmul(ps_nh[:, 0:NB], wt["W_inn_HB"][:], hb[:],
                                         start=False, stop=True)
                        nc.tensor.matmul(ps_nh[:, NB:2 * NB], wt["W_hn_H"][:],
                                         hg[0:96, :], start=True, stop=True)

                        srz = sbp.tile([96, 2 * NB], bf16, tag="SRZ")
                        nc.scalar.activation(srz[:, 0:NB], ps_rz[:, 0:NB], AF.Sigmoid,
                                             bias=wt["b_r96"][:, 0:1])
                        nc.scalar.activation(srz[:, NB:2 * NB], ps_rz[:, NB:2 * NB],
                                             AF.Sigmoid, bias=wt["b_z96"][:, 0:1])
                        t1 = sbp.tile([96, NB], f32, tag="t1")
                        nc.vector.scalar_tensor_tensor(t1[:], ps_nh[:, NB:2 * NB],
                                                       wt["b_hn96"][:, 0:1],
                                                       srz[:, 0:NB],
                                                       ALU.add, ALU.mult)
                        u = sbp.tile([96, NB], f32, tag="u")
                        nc.vector.scalar_tensor_tensor(u[:], ps_nh[:, 0:NB],
                                                       wt["b_inn96"][:, 0:1], t1[:],
                                                       ALU.add, ALU.add)
                        tn = sbp.tile([96, NB], bf16, tag="tn")
                        nc.scalar.activation(tn[:], u[:], AF.Tanh)
                        v = sbp.tile([96, NB], bf16, tag="v")
                        nc.vector.tensor_sub(v[:], hg[0:96, :], tn[:])
                        w2 = sbp.tile([96, NB], bf16, tag="w2")
                        nc.vector.tensor_mul(w2[:], srz[:, NB:2 * NB], v[:])
                        nc.vector.tensor_add(hg[0:96, :], tn[:], w2[:])

                # ---- BiLSTM over S steps (fw|bw paired along free dim) ----
                # XH [112, 2NB]: cols 0:NB fw, NB:2NB bw; rows 0:48 x, 48 ones
                # (bias row), 49:64 zero, 64:112 h.
                XH = lstmp.tile([112, 2 * NB], bf16, tag="XH", name=f"xh_{it}")
                C = lstmp.tile([48, 2 * NB], bf16, tag="C", name=f"c_{it}")
                HN = lstmp.tile([112, NB], bf16, tag="HN", name=f"hn_{it}")
                nc.sync.dma_start(XH[48:64, :], wt["ones16"][:, :])
                nc.vector.memset(XH[64:112, :], 0.0)
                nc.vector.memset(C[:], 0.0)
                nc.vector.memset(HN[32:64, :], 0.0)
                for t in range(S):
                    sf_, sb_ = t, 4 - t
                    # sb->sb DMA: partition offsets unconstrained (48-row slots)
                    nc.sync.dma_start(
                        XH[0:48, 0:NB],
                        HG[sf_ // 2][(sf_ % 2) * 48:(sf_ % 2) * 48 + 48, :])
                    nc.sync.dma_start(
                        XH[0:48, NB:2 * NB],
                        HG[sb_ // 2][(sb_ % 2) * 48:(sb_ % 2) * 48 + 48, :])
                    ps_c1 = psp.tile([112, 2 * NB], f32, tag="ps_rz",
                                     name=f"psc1_{it}_{t}")
                    ps_c2 = psp.tile([112, 2 * NB], f32, tag="ps_nh",
                                     name=f"psc2_{it}_{t}")
                    for cname, ps in (("c1", ps_c1), ("c2", ps_c2)):
                        for hh, dirn in ((0, "fw"), (1, "bw")):
                            nc.tensor.matmul(ps[:, hh * NB:(hh + 1) * NB],
                                             wt[f"L_{cname}_{dirn}"][:],
                                             XH[:, hh * NB:(hh + 1) * NB],
                                             start=True, stop=True)
                    sio = sbp.tile([112, 2 * NB], bf16, tag="sio")
                    nc.scalar.activation(sio[:], ps_c1[:], AF.Sigmoid)
                    tg = sbp.tile([48, 2 * NB], bf16, tag="tg")
                    nc.scalar.activation(tg[:], ps_c2[0:48, :], AF.Tanh)
                    sf2 = sbp.tile([48, 2 * NB], bf16, tag="sf2")
                    nc.scalar.activation(sf2[:], ps_c2[64:112, :], AF.Sigmoid)
                    t1l = sbp.tile([48, 2 * NB], bf16, tag="t1l")
                    nc.vector.tensor_mul(t1l[:], sio[0:48, :], tg[:])
                    t2l = sbp.tile([48, 2 * NB], bf16, tag="t2l")
                    nc.gpsimd.tensor_mul(t2l[:], sf2[:], C[:])
                    nc.vector.tensor_add(C[:], t1l[:], t2l[:])
                    tc2 = sbp.tile([112, 2 * NB], bf16, tag="tc2")
                    nc.scalar.activation(tc2[64:112, :], C[:], AF.Tanh)
                    if t < S - 1:
                        nc.vector.tensor_mul(XH[64:112, :], sio[64:112, :],
                                             tc2[64:112, :])
                    else:
                        nc.vector.tensor_mul(HN[0:48, :], sio[64:112, 0:NB],
                                             tc2[64:112, 0:NB])
                        nc.vector.tensor_mul(HN[64:112, :], sio[64:112, NB:2 * NB],
                                             tc2[64:112, NB:2 * NB])

                # ---- MLP ----
                psE = psp.tile([96, 2 * NB], f32, tag="ps_rz", name=f"psE_{it}")
                nc.tensor.matmul(psE[0:48, 0:NB], wt["We1"][:], HN[:],
                                 start=True, stop=True)
                e1 = sbp.tile([48, NB], bf16, tag="e1")
                nc.scalar.activation(e1[:], psE[0:48, 0:NB], AF.Relu,
                                     bias=wt["be1"][:, 0:1])
                psE2 = psp.tile([96, 2 * NB], f32, tag="ps_nh", name=f"psE2_{it}")
                nc.tensor.matmul(psE2[0:36, 0:NB], wt["We2"][:], e1[:],
                                 start=True, stop=True)
                e2 = sbp.tile([36, NB], bf16, tag="e2")
                nc.scalar.activation(e2[:], psE2[0:36, 0:NB], AF.Relu,
                                     bias=wt["be2"][:, 0:1])
                nc.tensor.matmul(psE[0:6, NB:2 * NB], wt["We3"][:], e2[:],
                                 start=True, stop=True)
                o = sbp.tile([6, NB], f32, tag="o")
                nc.scalar.activation(o[:], psE[0:6, NB:2 * NB], AF.Identity,
                                     bias=wt["be3"][:, 0:1])
                nc.sync.dma_start(out_d[:, c0:c0 + NB], o[:])

    split_excess_waits(nc)
    return nc


_NC_CACHE = None
TRACE = False
LAST_EXEC_NS = None


def kernel(nodes_feature, pos, attmat, **w):
    global _NC_CACHE, LAST_EXEC_NS
    from concourse.bass_utils import run_bass_kernel_spmd
    import concourse.mybir as mybir

    feat, att = prep_inputs(nodes_feature, pos, attmat)
    wts = build_weights(**w)

    if _NC_CACHE is None:
        _NC_CACHE = build_nc()
    nc = _NC_CACHE

    in_maps = []
    for c in range(NCORES):
        m = {"feat": np.ascontiguousarray(feat[:, :, c * NCORE:(c + 1) * NCORE]),
             "att": np.ascontiguousarray(att[:, :, c * NCORE:(c + 1) * NCORE])}
        for nm, shp, dt in WEIGHT_SPECS:
            m[nm] = wts[nm].astype(BF) if dt == mybir.dt.bfloat16 else wts[nm].astype(np.float32)
        in_maps.append(m)

    res = run_bass_kernel_spmd(nc, in_maps, core_ids=list(range(NCORES)),
                               trace=TRACE)
    LAST_EXEC_NS = res.exec_time_ns
    outs = [res.results[c]["out"] for c in range(NCORES)]     # [6, NCORE] each
    full = np.concatenate(outs, axis=1)                        # [6, N]
    return np.ascontiguousarray(full.T).astype(np.float32)     # [N, 6]


# revision 23
# speedup vs baseline: 1.0740x; 1.0740x over previous
"""Trainium2 Bass kernel for nn_Atomic_node_only_lstm (GNN message passing + BiLSTM + MLP).

v2: s-paired GRU with msg-linear folded into the gate matmuls.

Data-parallel over 8 NeuronCores (batch N=65536 -> 8192/core), NB=512 cols/tile.

Device layout (per batch tile of NB columns), seq steps grouped in pairs
(groups: (s0,s1), (s2,s3), (s4,zero)):
  H_g  [128, NB] bf16: rows 0:48 h_a, 48:96 h_b, 96:112 att_a, 112:128 att_b
       (h rows indexed k*12+d, k = node; att rows indexed n*4+w)
  arb_n [96, NB]: att[n,w] replicated over d, slot a rows 0:48, slot b 48:96
  Q_n  [96, NB] = arb_n * H_g[0:96]   (q_n[w*12+d] = att[n,w]*h[w,d], both slots)
  hbar [96, NB]: rows j*48+n*12+d = sum_w q over w   (slot j, node n)
  GRU gates from K=128 contraction of H_g (h part + att part folds s_n*(W_ih@msg_b))
  plus K=96 contraction of hbar (folds (W_ih@msg_w)).
  PSUM super-tiles [96, 1024] span 2 banks: rz = [r | z], nh = [inn | hn].

LSTM: XL [96, NB] = [x_fw | x_bw], HL [96, NB] = [h_fw | h_bw], C [96, NB].
  Per gate-type bank halves in free dim: B_if = [i | f], B_go = [g | o].
  x[j] = h[(k,d)], j = d*4+k (handled in lhsT construction).
MLP reads HL directly (en1_w cols 0:48 fw, 48:96 bw).
"""
import numpy as np
import ml_dtypes
from contextlib import ExitStack

N, S, K, D, H = 65536, 5, 4, 12, 48
NCORES = 8
NCORE = N // NCORES          # 8192 batch elements per core
NB = 512                     # batch columns per tile
NTILES = NCORE // NB
NGROUPS = 3                  # s-pairs: (0,1), (2,3), (4,-)

BF = ml_dtypes.bfloat16


# ----------------------------------------------------------------------------
# host-side weight construction (numpy, all tiny)
# ----------------------------------------------------------------------------
def build_weights(msg_w, msg_b, gru_w_ih, gru_w_hh, gru_b_ih, gru_b_hh,
                  lstm_w_ih_fw, lstm_w_hh_fw, lstm_b_ih_fw, lstm_b_hh_fw,
                  lstm_w_ih_bw, lstm_w_hh_bw, lstm_b_ih_bw, lstm_b_hh_bw,
                  en1_w, en1_b, en2_w, en2_b, en3_w, en3_b):
    out = {}
    # composed message->gate matrices [12 out-gate-d, 12 in-feat]
    A_r = gru_w_ih[0:12] @ msg_w      # gi_r = A_r @ hbar + s*(w_ih_r@msg_b)
    A_z = gru_w_ih[12:24] @ msg_w
    A_n = gru_w_ih[24:36] @ msg_w
    bi_r = gru_w_ih[0:12] @ msg_b     # [12]
    bi_z = gru_w_ih[12:24] @ msg_b
    bi_n = gru_w_ih[24:36] @ msg_b

    # RA_n lhsT [32, 96] (placed at partitions 96:128 on device; rhs = H_g[96:128]).
    # rows: 0:16 att_a (n*4+w), 16:32 att_b. cols: j*48 + w*12 + d.
    for n in range(4):
        R = np.zeros((32, 96), np.float32)
        for j in range(2):
            for w in range(4):
                for d in range(12):
                    R[j * 16 + n * 4 + w, j * 48 + w * 12 + d] = 1.0
        out[f"RA{n}"] = R

    # SR_n lhsT [96, 96]: contract Q_n -> hbar contribution of node n.
    # Q_n rows j*48 + w*12 + d ; hbar cols j*48 + n*12 + d.
    for n in range(4):
        Sm = np.zeros((96, 96), np.float32)
        for j in range(2):
            for w in range(4):
                for d in range(12):
                    Sm[j * 48 + w * 12 + d, j * 48 + n * 12 + d] = 1.0
        out[f"SR{n}"] = Sm

    # GRU gate lhsTs.
    # H-part [128, 96]: rhs = H_g (h rows j*48+k*12+dp, att rows 96+j*16+k*4+w)
    # out cols j*48 + k*12 + d.
    def gate_H(whh_blk, bi_blk):
        Wm = np.zeros((128, 96), np.float32)
        for j in range(2):
            for k in range(4):
                for d in range(12):
                    c = j * 48 + k * 12 + d
                    if whh_blk is not None:
                        for dp in range(12):
                            Wm[j * 48 + k * 12 + dp, c] = whh_blk[d, dp]
                    if bi_blk is not None:
                        for w in range(4):
                            Wm[96 + j * 16 + k * 4 + w, c] = bi_blk[d]
        return Wm

    def gate_HB(A_blk):
        Wm = np.zeros((96, 96), np.float32)
        for j in range(2):
            for k in range(4):
                for d in range(12):
                    c = j * 48 + k * 12 + d
                    for dp in range(12):
                        Wm[j * 48 + k * 12 + dp, c] = A_blk[d, dp]
        return Wm

    out["W_r_H"] = gate_H(gru_w_hh[0:12], bi_r)
    out["W_z_H"] = gate_H(gru_w_hh[12:24], bi_z)
    out["W_hn_H"] = gate_H(gru_w_hh[24:36], None)[0:96]   # h rows only, K=96
    out["W_inn_att"] = gate_H(None, bi_n)[96:128]         # att rows only, K=32
    out["W_r_HB"] = gate_HB(A_r)
    out["W_z_HB"] = gate_HB(A_z)
    out["W_inn_HB"] = gate_HB(A_n)

    out["b_r96"] = np.tile(gru_b_ih[0:12] + gru_b_hh[0:12], 8).reshape(96, 1).astype(np.float32)
    out["b_z96"] = np.tile(gru_b_ih[12:24] + gru_b_hh[12:24], 8).reshape(96, 1).astype(np.float32)
    out["b_inn96"] = np.tile(gru_b_ih[24:36], 8).reshape(96, 1).astype(np.float32)
    out["b_hn96"] = np.tile(gru_b_hh[24:36], 8).reshape(96, 1).astype(np.float32)
    # rz super-bias [96,1] applies to both halves? No - bias is per partition, free
    # dim halves share it. r and z biases differ -> separate ACT calls? No:
    # sigma(rz super-tile) is ONE act over [96, 1024]; bias per-partition only.
    # So we must fold r/z biases into the matmuls instead: add via att-part?
    # Simpler: bias rows are per-partition; r occupies cols 0:512 and z 512:1024 of
    # the SAME partitions -> per-partition bias cannot differ. Fold biases into
    # lhsT via the constant-1 trick: use att rows? att sums vary. Instead keep
    # two ACT calls when biases differ. For rz we instead ADD the bias inside the
    # H-part lhsT using a constant row... no constant row exists in H_g.
    # Resolution: biases b_r/b_z are added via activation bias -> need separate
    # sigma(r), sigma(z). To keep ONE act per super-tile we instead pre-add the
    # bias by augmenting att columns: s_n terms already use att rows; a constant
    # offset cannot come from data. So: two ACTs for rz after all (see kernel).

    # LSTM chunk lhsTs [112, 112]: rhs = XH half = [x(0:48) | ones-row(48) pad |
    # h(64:112)]; row 48 carries the bias (XH[48]=1, rows 49:64 zero).
    # c1 cols = [i(0:48) | gap | o(64:112)], c2 cols = [g(0:48) | gap | f(64:112)].
    chunks = {"c1": ("I", "O"), "c2": ("G", "F")}
    gidx = {"I": 0, "F": 1, "G": 2, "O": 3}
    wih = {"fw": lstm_w_ih_fw, "bw": lstm_w_ih_bw}
    whh = {"fw": lstm_w_hh_fw, "bw": lstm_w_hh_bw}
    bih = {"fw": lstm_b_ih_fw, "bw": lstm_b_ih_bw}
    bhh = {"fw": lstm_b_hh_fw, "bw": lstm_b_hh_bw}
    for cname, (ga, gb) in chunks.items():
        for dirn in ("fw", "bw"):
            M = np.zeros((112, 112), np.float32)
            for col0, gname in ((0, ga), (64, gb)):
                g = gidx[gname]
                wg = wih[dirn][g * 48:(g + 1) * 48, :]   # [48 out m, 48 xin jj]
                hg = whh[dirn][g * 48:(g + 1) * 48, :]
                for k in range(4):
                    for d in range(12):
                        M[k * 12 + d, col0:col0 + 48] = wg[:, d * 4 + k]
                M[48, col0:col0 + 48] = (bih[dirn][g * 48:(g + 1) * 48]
                                         + bhh[dirn][g * 48:(g + 1) * 48])
                M[64:112, col0:col0 + 48] = hg.T
            out[f"L_{cname}_{dirn}"] = M

    # MLP: HN = [h_fw(0:48) | gap | h_bw(64:112)]
    We1 = np.zeros((112, 48), np.float32)
    We1[0:48, :] = en1_w[:, 0:48].T
    We1[64:112, :] = en1_w[:, 48:96].T
    out["We1"] = We1
    out["be1"] = en1_b.reshape(48, 1).astype(np.float32)
    out["ones16"] = np.ones((16, 2 * NB), np.float32)
    out["We2"] = en2_w.T.copy()                 # [48, 36]
    out["be2"] = en2_b.reshape(36, 1).astype(np.float32)
    out["We3"] = en3_w.T.copy()                 # [36, 6]
    out["be3"] = en3_b.reshape(6, 1).astype(np.float32)
    return out


def prep_inputs(nodes_feature, pos, attmat):
    """Full-batch host layout: feat [S, 48, N] bf16 (k-major (k,d)), att [S, 16, N] bf16."""
    nf = np.concatenate([nodes_feature, pos], axis=-1)       # [N,S,K,12]
    feat = np.ascontiguousarray(nf.transpose(1, 2, 3, 0)).reshape(S, 48, N)
    att = np.ascontiguousarray(attmat.transpose(1, 2, 3, 0)).reshape(S, 16, N)
    return feat.astype(BF), att.astype(BF)


# ----------------------------------------------------------------------------
# device kernel builder
# ----------------------------------------------------------------------------
def split_excess_waits(nc, max_waits=1):
    import concourse.mybir as mybir
    cnt = 0
    for f in nc.m.functions:
        for bb in f.blocks:
            insts = bb.instructions
            new = []
            changed = False
            for inst in insts:
                si = inst.sync_info
                waits = list(si.on_wait) if si and si.on_wait else []
                if len(waits) > max_waits:
                    changed = True
                    k = 0
                    while len(waits) - k > max_waits:
                        chunk = waits[k:k + max_waits]
                        k += max_waits
                        cnt += 1
                        nop = mybir.InstNoOp(name=f"waitsplit-{cnt}", ins=[], outs=[])
                        nop.engine = inst.engine
                        nop.sync_info = mybir.SyncInfo(on_wait=chunk, on_update=[])
                        new.append(nop)
                    inst.sync_info = mybir.SyncInfo(
                        on_wait=waits[k:],
                        on_update=list(si.on_update) if si.on_update else [])
                new.append(inst)
            if changed:
                bb.instructions = new
    return cnt


WEIGHT_SPECS = None  # filled in build_nc


def build_nc():
    import concourse.bass as bass
    import concourse.tile as tile
    from concourse import mybir

    f32 = mybir.dt.float32
    bf16 = mybir.dt.bfloat16
    AF = mybir.ActivationFunctionType
    ALU = mybir.AluOpType

    nc = bass.Bass("TRN2")

    feat_d = nc.dram_tensor("feat", [S, 48, NCORE], bf16, kind="ExternalInput")
    att_d = nc.dram_tensor("att", [S, 16, NCORE], bf16, kind="ExternalInput")

    wspecs = []
    for n in range(4):
        wspecs.append((f"RA{n}", (32, 96), bf16))
        wspecs.append((f"SR{n}", (96, 96), bf16))
    wspecs += [("W_r_H", (128, 96), bf16), ("W_z_H", (128, 96), bf16),
               ("W_hn_H", (96, 96), bf16), ("W_inn_att", (32, 96), bf16),
               ("W_r_HB", (96, 96), bf16), ("W_z_HB", (96, 96), bf16),
               ("W_inn_HB", (96, 96), bf16),
               ("b_r96", (96, 1), f32), ("b_z96", (96, 1), f32),
               ("b_inn96", (96, 1), f32), ("b_hn96", (96, 1), f32)]
    for cname in ("c1", "c2"):
        for dirn in ("fw", "bw"):
            wspecs.append((f"L_{cname}_{dirn}", (112, 112), bf16))
    wspecs.append(("ones16", (16, 2 * NB), bf16))
    wspecs += [("We1", (112, 48), bf16), ("be1", (48, 1), f32),
               ("We2", (48, 36), bf16), ("be2", (36, 1), f32),
               ("We3", (36, 6), bf16), ("be3", (6, 1), f32)]

    wnames = {}
    for nm, shp, dt in wspecs:
        wnames[nm] = nc.dram_tensor(nm, list(shp), dt, kind="ExternalInput")
    out_d = nc.dram_tensor("out", [6, NCORE], f32, kind="ExternalOutput")
    RA_QUAD = {0: 0, 1: 32, 2: 64, 3: 96}  # row-quadrant per RA matmul

    global WEIGHT_SPECS
    WEIGHT_SPECS = wspecs

    with tile.TileContext(nc) as tc:
        with ExitStack() as ctx:
            wpool = ctx.enter_context(tc.tile_pool(name="weights", bufs=1))
            wt = {}
            for nm, shp, dt in wspecs:
                # lhsT base partition must match the rhs row quadrant:
                # RA{n} runs at row quadrant 32n; W_inn_att at quadrant 96.
                if nm.startswith("RA"):
                    q = RA_QUAD[int(nm[2])]
                    t = wpool.tile([128, shp[1]], dt, tag=f"w_{nm}")
                    nc.sync.dma_start(t[q:q + 32, :], wnames[nm][:])
                    wt[nm] = t
                elif nm == "W_inn_att":
                    t = wpool.tile([128, shp[1]], dt, tag=f"w_{nm}")
                    nc.sync.dma_start(t[96:128, :], wnames[nm][:])
                    wt[nm] = t
                else:
                    t = wpool.tile([shp[0], shp[1]], dt, tag=f"w_{nm}")
                    nc.sync.dma_start(t[:], wnames[nm][:])
                    wt[nm] = t

            hpool = ctx.enter_context(tc.tile_pool(name="hg", bufs=2))
            arpool = ctx.enter_context(tc.tile_pool(name="ar", bufs=2))
            sbp = ctx.enter_context(tc.tile_pool(name="work", bufs=4))
            lstmp = ctx.enter_context(tc.tile_pool(name="lstm", bufs=2))
            psp = ctx.enter_context(tc.tile_pool(name="ps", bufs=2, space="PSUM"))

            for it in range(NTILES):
                c0 = it * NB
                # ---- load groups ----
                HG = []
                for g in range(NGROUPS):
                    hg = hpool.tile([128, NB], bf16, tag=f"HG{g}")
                    sa = 2 * g
                    if g == 2:
                        # zero slot-b rows first (partition offsets must be 32-aligned;
                        # DMAs below restore the overlapping real rows)
                        nc.vector.memset(hg[32:64, :], 0.0)
                        nc.vector.memset(hg[64:96, :], 0.0)
                        nc.vector.memset(hg[96:128, :], 0.0)
                    nc.sync.dma_start(hg[0:48, :], feat_d[sa, :, c0:c0 + NB])
                    nc.sync.dma_start(hg[96:112, :], att_d[sa, :, c0:c0 + NB])
                    if g < 2:
                        sb = 2 * g + 1
                        nc.sync.dma_start(hg[48:96, :], feat_d[sb, :, c0:c0 + NB])
                        nc.sync.dma_start(hg[112:128, :], att_d[sb, :, c0:c0 + NB])
                    HG.append(hg)

                # att replicated into all 4 row-quadrants so the 4 RA matmuls can
                # run concurrently in distinct PE row groups (quadrant 3 = hg).
                AT4 = []
                for g in range(NGROUPS):
                    at4 = hpool.tile([96, NB], bf16, tag=f"AT4{g}")
                    if g == 2:
                        nc.vector.memset(at4[:], 0.0)
                    for q in range(3):
                        nc.sync.dma_start(at4[32 * q:32 * q + 16, :],
                                          att_d[2 * g, :, c0:c0 + NB])
                        if g < 2:
                            nc.sync.dma_start(at4[32 * q + 16:32 * q + 32, :],
                                              att_d[2 * g + 1, :, c0:c0 + NB])
                    AT4.append(at4)

                # ---- attention replication for all groups ----
                ARBG = {}
                for g in range(NGROUPS):
                    hg = HG[g]
                    ps_a = psp.tile([96, 2 * NB], f32, tag="ps_rz", name=f"psa_{it}_{g}")
                    ps_b = psp.tile([96, 2 * NB], f32, tag="ps_nh", name=f"psb_{it}_{g}")
                    for n in range(4):
                        ps = (ps_a, ps_b)[n // 2]
                        sl = ps[:, (n % 2) * NB:(n % 2 + 1) * NB]
                        q = RA_QUAD[n]
                        rhs = (hg[96:128, :] if n == 3
                               else AT4[g][q:q + 32, :])
                        nc.tensor.matmul(sl, wt[f"RA{n}"][q:q + 32, :], rhs,
                                         start=True, stop=True,
                                         tile_position=(q, 0))
                        ar = arpool.tile([96, NB], bf16, tag=f"ARB{g}_{n}")
                        if n % 2 == 0:
                            nc.vector.tensor_copy(ar[:], sl)
                        else:
                            nc.scalar.copy(ar[:], sl)
                        ARBG[(g, n)] = ar

                # ---- 2 GRU passes, groups interleaved (3 independent chains) ----
                for pas in range(2):
                    for g in range(NGROUPS):
                        hg = HG[g]
                        ARB = [ARBG[(g, n)] for n in range(4)]
                        # Q_n = arb_n * h (both slots at once)
                        Q = []
                        for n in range(4):
                            q = sbp.tile([96, NB], bf16, tag=f"Q{n}")
                            if n >= 2:
                                nc.gpsimd.tensor_tensor(q[:], ARB[n][:], hg[0:96, :],
                                                        ALU.mult)
                            else:
                                nc.vector.tensor_mul(q[:], ARB[n][:], hg[0:96, :])
                            Q.append(q)
                        # hbar = sum_n SR_n @ Q_n
                        ps_hb = psp.tile([96, 2 * NB], f32, tag="ps_nh",
                                         name=f"pshb_{it}_{g}_{pas}")
                        for n in range(4):
                            nc.tensor.matmul(ps_hb[:, 0:NB], wt[f"SR{n}"][:], Q[n][:],
                                             start=(n == 0), stop=(n == 3))
                        hb = sbp.tile([96, NB], bf16, tag="HBs")
                        nc.vector.tensor_copy(hb[:], ps_hb[:, 0:NB])

                        # gates
                        ps_rz = psp.tile([96, 2 * NB], f32, tag="ps_rz",
                                         name=f"psrz_{it}_{g}_{pas}")
                        nc.tensor.matmul(ps_rz[:, 0:NB], wt["W_r_H"][:], hg[0:128, :],
                                         start=True, stop=False)
                        nc.tensor.matmul(ps_rz[:, 0:NB], wt["W_r_HB"][:], hb[:],
                                         start=False, stop=True)
                        nc.tensor.matmul(ps_rz[:, NB:2 * NB], wt["W_z_H"][:], hg[0:128, :],
                                         start=True, stop=False)
                        nc.tensor.matmul(ps_rz[:, NB:2 * NB], wt["W_z_HB"][:], hb[:],
                                         start=False, stop=True)
                        ps_nh = psp.tile([96, 2 * NB], f32, tag="ps_nh",
                                         name=f"psnh_{it}_{g}_{pas}")
                        nc.tensor.matmul(ps_nh[:, 0:NB], wt["W_inn_att"][96:128, :],
                                         hg[96:128, :], start=True, stop=False,
                                         tile_position=(96, 0))
                        nc.tensor.matmul(ps_nh[:, 0:NB], wt["W_inn_HB"][:], hb[:],
                                         start=False, stop=True)
                        nc.tensor.matmul(ps_nh[:, NB:2 * NB], wt["W_hn_H"][:],
                                         hg[0:96, :], start=True, stop=True)

                        srz = sbp.tile([96, 2 * NB], bf16, tag="SRZ")
                        nc.scalar.activation(srz[:, 0:NB], ps_rz[:, 0:NB], AF.Sigmoid,
                                             bias=wt["b_r96"][:, 0:1])
                        nc.scalar.activation(srz[:, NB:2 * NB], ps_rz[:, NB:2 * NB],
                                             AF.Sigmoid, bias=wt["b_z96"][:, 0:1])
                        t1 = sbp.tile([96, NB], f32, tag="t1")
                        nc.vector.scalar_tensor_tensor(t1[:], ps_nh[:, NB:2 * NB],
                                                       wt["b_hn96"][:, 0:1],
                                                       srz[:, 0:NB],
                                                       ALU.add, ALU.mult)
                        u = sbp.tile([96, NB], f32, tag="u")
                        nc.vector.scalar_tensor_tensor(u[:], ps_nh[:, 0:NB],
                                                       wt["b_inn96"][:, 0:1], t1[:],
                                                       ALU.add, ALU.add)
                        tn = sbp.tile([96, NB], bf16, tag="tn")
                        nc.scalar.activation(tn[:], u[:], AF.Tanh)
                        v = sbp.tile([96, NB], bf16, tag="v")
                        nc.vector.tensor_sub(v[:], hg[0:96, :], tn[:])
                        w2 = sbp.tile([96, NB], bf16, tag="w2")
                        nc.vector.tensor_mul(w2[:], srz[:, NB:2 * NB], v[:])
                        nc.vector.tensor_add(hg[0:96, :], tn[:], w2[:])

                # ---- BiLSTM over S steps (fw|bw paired along free dim) ----
                # XH_t [112, 2NB]: cols 0:NB fw, NB:2NB bw; rows 0:48 x, 48 ones
                # (bias row), 49:64 zero, 64:112 h. One tile per step, staged a
                # step ahead so the x DMAs are off the recurrence path.
                def stage_xh(t):
                    xh = lstmp.tile([112, 2 * NB], bf16, tag=f"XH{t % 2}",
                                    name=f"xh_{it}_{t}")
                    sft, sbt = t, 4 - t
                    nc.sync.dma_start(xh[48:64, :], wt["ones16"][:, :])
                    nc.sync.dma_start(
                        xh[0:48, 0:NB],
                        HG[sft // 2][(sft % 2) * 48:(sft % 2) * 48 + 48, :])
                    nc.sync.dma_start(
                        xh[0:48, NB:2 * NB],
                        HG[sbt // 2][(sbt % 2) * 48:(sbt % 2) * 48 + 48, :])
                    return xh

                C = lstmp.tile([48, 2 * NB], bf16, tag="C", name=f"c_{it}")
                HN = lstmp.tile([112, NB], bf16, tag="HN", name=f"hn_{it}")
                nc.vector.memset(C[:], 0.0)
                nc.vector.memset(HN[32:64, :], 0.0)
                XH = stage_xh(0)
                nc.vector.memset(XH[64:112, :], 0.0)
                for t in range(S):
                    ps_c1 = psp.tile([112, 2 * NB], f32, tag="ps_rz",
                                     name=f"psc1_{it}_{t}")
                    ps_c2 = psp.tile([112, 2 * NB], f32, tag="ps_nh",
                                     name=f"psc2_{it}_{t}")
                    for cname, ps in (("c1", ps_c1), ("c2", ps_c2)):
                        for hh, dirn in ((0, "fw"), (1, "bw")):
                            nc.tensor.matmul(ps[:, hh * NB:(hh + 1) * NB],
                                             wt[f"L_{cname}_{dirn}"][:],
                                             XH[:, hh * NB:(hh + 1) * NB],
                                             start=True, stop=True)
                    XHn = stage_xh(t + 1) if t < S - 1 else None
                    sio = sbp.tile([112, 2 * NB], bf16, tag="sio")
                    nc.scalar.activation(sio[:], ps_c1[:], AF.Sigmoid)
                    tg = sbp.tile([48, 2 * NB], bf16, tag="tg")
                    nc.scalar.activation(tg[:], ps_c2[0:48, :], AF.Tanh)
                    sf2 = sbp.tile([48, 2 * NB], bf16, tag="sf2")
                    nc.scalar.activation(sf2[:], ps_c2[64:112, :], AF.Sigmoid)
                    t1l = sbp.tile([48, 2 * NB], bf16, tag="t1l")
                    nc.vector.tensor_mul(t1l[:], sio[0:48, :], tg[:])
                    t2l = sbp.tile([48, 2 * NB], bf16, tag="t2l")
                    nc.vector.tensor_mul(t2l[:], sf2[:], C[:])
                    nc.vector.tensor_add(C[:], t1l[:], t2l[:])
                    tc2 = sbp.tile([112, 2 * NB], bf16, tag="tc2")
                    nc.scalar.activation(tc2[64:112, :], C[:], AF.Tanh)
                    if t < S - 1:
                        nc.vector.tensor_mul(XHn[64:112, :], sio[64:112, :],
                                             tc2[64:112, :])
                        XH = XHn
                    else:
                        nc.vector.tensor_mul(HN[0:48, :], sio[64:112, 0:NB],
                                             tc2[64:112, 0:NB])
                        nc.vector.tensor_mul(HN[64:112, :], sio[64:112, NB:2 * NB],
                                             tc2[64:112, NB:2 * NB])

                # ---- MLP ----
                psE = psp.tile([96, 2 * NB], f32, tag="ps_rz", name=f"psE_{it}")
                nc.tensor.matmul(psE[0:48, 0:NB], wt["We1"][:], HN[:],
                                 start=True, stop=True)
                e1 = sbp.tile([48, NB], bf16, tag="e1")
                nc.scalar.activation(e1[:], psE[0:48, 0:NB], AF.Relu,
                                     bias=wt["be1"][:, 0:1])
                psE2 = psp.tile([96, 2 * NB], f32, tag="ps_nh", name=f"psE2_{it}")
                nc.tensor.matmul(psE2[0:36, 0:NB], wt["We2"][:], e1[:],
                                 start=True, stop=True)
                e2 = sbp.tile([36, NB], bf16, tag="e2")
                nc.scalar.activation(e2[:], psE2[0:36, 0:NB], AF.Relu,
                                     bias=wt["be2"][:, 0:1])
                nc.tensor.matmul(psE[0:6, NB:2 * NB], wt["We3"][:], e2[:],
                                 start=True, stop=True)
                o = sbp.tile([6, NB], f32, tag="o")
                nc.scalar.activation(o[:], psE[0:6, NB:2 * NB], AF.Identity,
                                     bias=wt["be3"][:, 0:1])
                nc.sync.dma_start(out_d[:, c0:c0 + NB], o[:])

    split_excess_waits(nc)
    return nc


_NC_CACHE = None
TRACE = False
LAST_EXEC_NS = None


def kernel(nodes_feature, pos, attmat, **w):
    global _NC_CACHE, LAST_EXEC_NS
    from concourse.bass_utils import run_bass_kernel_spmd
    import concourse.mybir as mybir

    feat, att = prep_inputs(nodes_feature, pos, attmat)
    wts = build_weights(**w)

    if _NC_CACHE is None:
        _NC_CACHE = build_nc()
    nc = _NC_CACHE

    in_maps = []
    for c in range(NCORES):
        m = {"feat": np.ascontiguousarray(feat[:, :, c * NCORE:(c + 1) * NCORE]),
             "att": np.ascontiguousarray(att[:, :, c * NCORE:(c + 1) * NCORE])}
        for nm, shp, dt in WEIGHT_SPECS:
            m[nm] = wts[nm].astype(BF) if dt == mybir.dt.bfloat16 else wts[nm].astype(np.float32)
        in_maps.append(m)

    res = run_bass_kernel_spmd(nc, in_maps, core_ids=list(range(NCORES)),
                               trace=TRACE)
    LAST_EXEC_NS = res.exec_time_ns
    outs = [res.results[c]["out"] for c in range(NCORES)]     # [6, NCORE] each
    full = np.concatenate(outs, axis=1)                        # [6, N]
    return np.ascontiguousarray(full.T).astype(np.float32)     # [N, 6]


# revision 24
# speedup vs baseline: 1.0771x; 1.0028x over previous
"""Trainium2 Bass kernel for nn_Atomic_node_only_lstm (GNN message passing + BiLSTM + MLP).

v2: s-paired GRU with msg-linear folded into the gate matmuls.

Data-parallel over 8 NeuronCores (batch N=65536 -> 8192/core), NB=512 cols/tile.

Device layout (per batch tile of NB columns), seq steps grouped in pairs
(groups: (s0,s1), (s2,s3), (s4,zero)):
  H_g  [128, NB] bf16: rows 0:48 h_a, 48:96 h_b, 96:112 att_a, 112:128 att_b
       (h rows indexed k*12+d, k = node; att rows indexed n*4+w)
  arb_n [96, NB]: att[n,w] replicated over d, slot a rows 0:48, slot b 48:96
  Q_n  [96, NB] = arb_n * H_g[0:96]   (q_n[w*12+d] = att[n,w]*h[w,d], both slots)
  hbar [96, NB]: rows j*48+n*12+d = sum_w q over w   (slot j, node n)
  GRU gates from K=128 contraction of H_g (h part + att part folds s_n*(W_ih@msg_b))
  plus K=96 contraction of hbar (folds (W_ih@msg_w)).
  PSUM super-tiles [96, 1024] span 2 banks: rz = [r | z], nh = [inn | hn].

LSTM: XL [96, NB] = [x_fw | x_bw], HL [96, NB] = [h_fw | h_bw], C [96, NB].
  Per gate-type bank halves in free dim: B_if = [i | f], B_go = [g | o].
  x[j] = h[(k,d)], j = d*4+k (handled in lhsT construction).
MLP reads HL directly (en1_w cols 0:48 fw, 48:96 bw).
"""
import numpy as np
import ml_dtypes
from contextlib import ExitStack

N, S, K, D, H = 65536, 5, 4, 12, 48
NCORES = 8
NCORE = N // NCORES          # 8192 batch elements per core
NB = 512                     # batch columns per tile
NTILES = NCORE // NB
NGROUPS = 3                  # s-pairs: (0,1), (2,3), (4,-)

BF = ml_dtypes.bfloat16


# ----------------------------------------------------------------------------
# host-side weight construction (numpy, all tiny)
# ----------------------------------------------------------------------------
def build_weights(msg_w, msg_b, gru_w_ih, gru_w_hh, gru_b_ih, gru_b_hh,
                  lstm_w_ih_fw, lstm_w_hh_fw, lstm_b_ih_fw, lstm_b_hh_fw,
                  lstm_w_ih_bw, lstm_w_hh_bw, lstm_b_ih_bw, lstm_b_hh_bw,
                  en1_w, en1_b, en2_w, en2_b, en3_w, en3_b):
    out = {}
    # composed message->gate matrices [12 out-gate-d, 12 in-feat]
    A_r = gru_w_ih[0:12] @ msg_w      # gi_r = A_r @ hbar + s*(w_ih_r@msg_b)
    A_z = gru_w_ih[12:24] @ msg_w
    A_n = gru_w_ih[24:36] @ msg_w
    bi_r = gru_w_ih[0:12] @ msg_b     # [12]
    bi_z = gru_w_ih[12:24] @ msg_b
    bi_n = gru_w_ih[24:36] @ msg_b

    # RA_n lhsT [32, 96] (placed at partitions 96:128 on device; rhs = H_g[96:128]).
    # rows: 0:16 att_a (n*4+w), 16:32 att_b. cols: j*48 + w*12 + d.
    for n in range(4):
        R = np.zeros((32, 96), np.float32)
        for j in range(2):
            for w in range(4):
                for d in range(12):
                    R[j * 16 + n * 4 + w, j * 48 + w * 12 + d] = 1.0
        out[f"RA{n}"] = R

    # SR_n lhsT [96, 96]: contract Q_n -> hbar contribution of node n.
    # Q_n rows j*48 + w*12 + d ; hbar cols j*48 + n*12 + d.
    for n in range(4):
        Sm = np.zeros((96, 96), np.float32)
        for j in range(2):
            for w in range(4):
                for d in range(12):
                    Sm[j * 48 + w * 12 + d, j * 48 + n * 12 + d] = 1.0
        out[f"SR{n}"] = Sm

    # GRU gate lhsTs.
    # H-part [128, 96]: rhs = H_g (h rows j*48+k*12+dp, att rows 96+j*16+k*4+w)
    # out cols j*48 + k*12 + d.
    def gate_H(whh_blk, bi_blk):
        Wm = np.zeros((128, 96), np.float32)
        for j in range(2):
            for k in range(4):
                for d in range(12):
                    c = j * 48 + k * 12 + d
                    if whh_blk is not None:
                        for dp in range(12):
                            Wm[j * 48 + k * 12 + dp, c] = whh_blk[d, dp]
                    if bi_blk is not None:
                        for w in range(4):
                            Wm[96 + j * 16 + k * 4 + w, c] = bi_blk[d]
        return Wm

    def gate_HB(A_blk):
        Wm = np.zeros((96, 96), np.float32)
        for j in range(2):
            for k in range(4):
                for d in range(12):
                    c = j * 48 + k * 12 + d
                    for dp in range(12):
                        Wm[j * 48 + k * 12 + dp, c] = A_blk[d, dp]
        return Wm

    out["W_r_H"] = gate_H(gru_w_hh[0:12], bi_r)
    out["W_z_H"] = gate_H(gru_w_hh[12:24], bi_z)
    out["W_hn_H"] = gate_H(gru_w_hh[24:36], None)[0:96]   # h rows only, K=96
    out["W_inn_att"] = gate_H(None, bi_n)[96:128]         # att rows only, K=32
    out["W_r_HB"] = gate_HB(A_r)
    out["W_z_HB"] = gate_HB(A_z)
    out["W_inn_HB"] = gate_HB(A_n)

    out["b_r96"] = np.tile(gru_b_ih[0:12] + gru_b_hh[0:12], 8).reshape(96, 1).astype(np.float32)
    out["b_z96"] = np.tile(gru_b_ih[12:24] + gru_b_hh[12:24], 8).reshape(96, 1).astype(np.float32)
    out["b_inn96"] = np.tile(gru_b_ih[24:36], 8).reshape(96, 1).astype(np.float32)
    out["b_hn96"] = np.tile(gru_b_hh[24:36], 8).reshape(96, 1).astype(np.float32)
    # rz super-bias [96,1] applies to both halves? No - bias is per partition, free
    # dim halves share it. r and z biases differ -> separate ACT calls? No:
    # sigma(rz super-tile) is ONE act over [96, 1024]; bias per-partition only.
    # So we must fold r/z biases into the matmuls instead: add via att-part?
    # Simpler: bias rows are per-partition; r occupies cols 0:512 and z 512:1024 of
    # the SAME partitions -> per-partition bias cannot differ. Fold biases into
    # lhsT via the constant-1 trick: use att rows? att sums vary. Instead keep
    # two ACT calls when biases differ. For rz we instead ADD the bias inside the
    # H-part lhsT using a constant row... no constant row exists in H_g.
    # Resolution: biases b_r/b_z are added via activation bias -> need separate
    # sigma(r), sigma(z). To keep ONE act per super-tile we instead pre-add the
    # bias by augmenting att columns: s_n terms already use att rows; a constant
    # offset cannot come from data. So: two ACTs for rz after all (see kernel).

    # LSTM chunk lhsTs [112, 112]: rhs = XH half = [x(0:48) | ones-row(48) pad |
    # h(64:112)]; row 48 carries the bias (XH[48]=1, rows 49:64 zero).
    # c1 cols = [i(0:48) | gap | o(64:112)], c2 cols = [g(0:48) | gap | f(64:112)].
    chunks = {"c1": ("I", "O"), "c2": ("G", "F")}
    gidx = {"I": 0, "F": 1, "G": 2, "O": 3}
    wih = {"fw": lstm_w_ih_fw, "bw": lstm_w_ih_bw}
    whh = {"fw": lstm_w_hh_fw, "bw": lstm_w_hh_bw}
    bih = {"fw": lstm_b_ih_fw, "bw": lstm_b_ih_bw}
    bhh = {"fw": lstm_b_hh_fw, "bw": lstm_b_hh_bw}
    for cname, (ga, gb) in chunks.items():
        for dirn in ("fw", "bw"):
            M = np.zeros((112, 112), np.float32)
            for col0, gname in ((0, ga), (64, gb)):
                g = gidx[gname]
                wg = wih[dirn][g * 48:(g + 1) * 48, :]   # [48 out m, 48 xin jj]
                hg = whh[dirn][g * 48:(g + 1) * 48, :]
                for k in range(4):
                    for d in range(12):
                        M[k * 12 + d, col0:col0 + 48] = wg[:, d * 4 + k]
                M[48, col0:col0 + 48] = (bih[dirn][g * 48:(g + 1) * 48]
                                         + bhh[dirn][g * 48:(g + 1) * 48])
                M[64:112, col0:col0 + 48] = hg.T
            out[f"L_{cname}_{dirn}"] = M

    # MLP: HN = [h_fw(0:48) | gap | h_bw(64:112)]
    We1 = np.zeros((112, 48), np.float32)
    We1[0:48, :] = en1_w[:, 0:48].T
    We1[64:112, :] = en1_w[:, 48:96].T
    out["We1"] = We1
    out["be1"] = en1_b.reshape(48, 1).astype(np.float32)
    out["ones16"] = np.ones((16, 2 * NB), np.float32)
    out["We2"] = en2_w.T.copy()                 # [48, 36]
    out["be2"] = en2_b.reshape(36, 1).astype(np.float32)
    out["We3"] = en3_w.T.copy()                 # [36, 6]
    out["be3"] = en3_b.reshape(6, 1).astype(np.float32)
    return out


def prep_inputs(nodes_feature, pos, attmat):
    """Full-batch host layout: feat [S, 48, N] bf16 (k-major (k,d)), att [S, 16, N] bf16."""
    nf = np.concatenate([nodes_feature, pos], axis=-1)       # [N,S,K,12]
    feat = np.ascontiguousarray(nf.transpose(1, 2, 3, 0)).reshape(S, 48, N)
    att = np.ascontiguousarray(attmat.transpose(1, 2, 3, 0)).reshape(S, 16, N)
    return feat.astype(BF), att.astype(BF)


# ----------------------------------------------------------------------------
# device kernel builder
# ----------------------------------------------------------------------------
def split_excess_waits(nc, max_waits=1):
    import concourse.mybir as mybir
    cnt = 0
    for f in nc.m.functions:
        for bb in f.blocks:
            insts = bb.instructions
            new = []
            changed = False
            for inst in insts:
                si = inst.sync_info
                waits = list(si.on_wait) if si and si.on_wait else []
                if len(waits) > max_waits:
                    changed = True
                    k = 0
                    while len(waits) - k > max_waits:
                        chunk = waits[k:k + max_waits]
                        k += max_waits
                        cnt += 1
                        nop = mybir.InstNoOp(name=f"waitsplit-{cnt}", ins=[], outs=[])
                        nop.engine = inst.engine
                        nop.sync_info = mybir.SyncInfo(on_wait=chunk, on_update=[])
                        new.append(nop)
                    inst.sync_info = mybir.SyncInfo(
                        on_wait=waits[k:],
                        on_update=list(si.on_update) if si.on_update else [])
                new.append(inst)
            if changed:
                bb.instructions = new
    return cnt


WEIGHT_SPECS = None  # filled in build_nc


def build_nc():
    import concourse.bass as bass
    import concourse.tile as tile
    from concourse import mybir

    f32 = mybir.dt.float32
    bf16 = mybir.dt.bfloat16
    AF = mybir.ActivationFunctionType
    ALU = mybir.AluOpType

    nc = bass.Bass("TRN2")

    feat_d = nc.dram_tensor("feat", [S, 48, NCORE], bf16, kind="ExternalInput")
    att_d = nc.dram_tensor("att", [S, 16, NCORE], bf16, kind="ExternalInput")

    wspecs = []
    for n in range(4):
        wspecs.append((f"RA{n}", (32, 96), bf16))
        wspecs.append((f"SR{n}", (96, 96), bf16))
    wspecs += [("W_r_H", (128, 96), bf16), ("W_z_H", (128, 96), bf16),
               ("W_hn_H", (96, 96), bf16), ("W_inn_att", (32, 96), bf16),
               ("W_r_HB", (96, 96), bf16), ("W_z_HB", (96, 96), bf16),
               ("W_inn_HB", (96, 96), bf16),
               ("b_r96", (96, 1), f32), ("b_z96", (96, 1), f32),
               ("b_inn96", (96, 1), f32), ("b_hn96", (96, 1), f32)]
    for cname in ("c1", "c2"):
        for dirn in ("fw", "bw"):
            wspecs.append((f"L_{cname}_{dirn}", (112, 112), bf16))
    wspecs.append(("ones16", (16, 2 * NB), bf16))
    wspecs += [("We1", (112, 48), bf16), ("be1", (48, 1), f32),
               ("We2", (48, 36), bf16), ("be2", (36, 1), f32),
               ("We3", (36, 6), bf16), ("be3", (6, 1), f32)]

    wnames = {}
    for nm, shp, dt in wspecs:
        wnames[nm] = nc.dram_tensor(nm, list(shp), dt, kind="ExternalInput")
    out_d = nc.dram_tensor("out", [6, NCORE], f32, kind="ExternalOutput")
    RA_QUAD = {0: 0, 1: 32, 2: 64, 3: 96}  # row-quadrant per RA matmul

    global WEIGHT_SPECS
    WEIGHT_SPECS = wspecs

    with tile.TileContext(nc) as tc:
        with ExitStack() as ctx:
            wpool = ctx.enter_context(tc.tile_pool(name="weights", bufs=1))
            wt = {}
            for nm, shp, dt in wspecs:
                # lhsT base partition must match the rhs row quadrant:
                # RA{n} runs at row quadrant 32n; W_inn_att at quadrant 96.
                if nm.startswith("RA"):
                    q = RA_QUAD[int(nm[2])]
                    t = wpool.tile([128, shp[1]], dt, tag=f"w_{nm}")
                    nc.sync.dma_start(t[q:q + 32, :], wnames[nm][:])
                    wt[nm] = t
                elif nm == "W_inn_att":
                    t = wpool.tile([128, shp[1]], dt, tag=f"w_{nm}")
                    nc.sync.dma_start(t[96:128, :], wnames[nm][:])
                    wt[nm] = t
                else:
                    t = wpool.tile([shp[0], shp[1]], dt, tag=f"w_{nm}")
                    nc.sync.dma_start(t[:], wnames[nm][:])
                    wt[nm] = t

            hpool = ctx.enter_context(tc.tile_pool(name="hg", bufs=3))
            arpool = ctx.enter_context(tc.tile_pool(name="ar", bufs=3))
            sbp = ctx.enter_context(tc.tile_pool(name="work", bufs=4))
            lstmp = ctx.enter_context(tc.tile_pool(name="lstm", bufs=3))
            psp = ctx.enter_context(tc.tile_pool(name="ps", bufs=2, space="PSUM"))

            for it in range(NTILES):
                c0 = it * NB
                # ---- load groups ----
                HG = []
                for g in range(NGROUPS):
                    hg = hpool.tile([128, NB], bf16, tag=f"HG{g}")
                    sa = 2 * g
                    if g == 2:
                        # zero slot-b rows first (partition offsets must be 32-aligned;
                        # DMAs below restore the overlapping real rows)
                        nc.vector.memset(hg[32:64, :], 0.0)
                        nc.vector.memset(hg[64:96, :], 0.0)
                        nc.vector.memset(hg[96:128, :], 0.0)
                    nc.sync.dma_start(hg[0:48, :], feat_d[sa, :, c0:c0 + NB])
                    nc.sync.dma_start(hg[96:112, :], att_d[sa, :, c0:c0 + NB])
                    if g < 2:
                        sb = 2 * g + 1
                        nc.sync.dma_start(hg[48:96, :], feat_d[sb, :, c0:c0 + NB])
                        nc.sync.dma_start(hg[112:128, :], att_d[sb, :, c0:c0 + NB])
                    HG.append(hg)

                # att replicated into all 4 row-quadrants so the 4 RA matmuls can
                # run concurrently in distinct PE row groups (quadrant 3 = hg).
                AT4 = []
                for g in range(NGROUPS):
                    at4 = hpool.tile([96, NB], bf16, tag=f"AT4{g}")
                    if g == 2:
                        nc.vector.memset(at4[:], 0.0)
                    for q in range(3):
                        nc.sync.dma_start(at4[32 * q:32 * q + 16, :],
                                          att_d[2 * g, :, c0:c0 + NB])
                        if g < 2:
                            nc.sync.dma_start(at4[32 * q + 16:32 * q + 32, :],
                                              att_d[2 * g + 1, :, c0:c0 + NB])
                    AT4.append(at4)

                # ---- attention replication for all groups ----
                ARBG = {}
                for g in range(NGROUPS):
                    hg = HG[g]
                    ps_a = psp.tile([96, 2 * NB], f32, tag="ps_rz", name=f"psa_{it}_{g}")
                    ps_b = psp.tile([96, 2 * NB], f32, tag="ps_nh", name=f"psb_{it}_{g}")
                    for n in range(4):
                        ps = (ps_a, ps_b)[n // 2]
                        sl = ps[:, (n % 2) * NB:(n % 2 + 1) * NB]
                        q = RA_QUAD[n]
                        rhs = (hg[96:128, :] if n == 3
                               else AT4[g][q:q + 32, :])
                        nc.tensor.matmul(sl, wt[f"RA{n}"][q:q + 32, :], rhs,
                                         start=True, stop=True,
                                         tile_position=(q, 0))
                        ar = arpool.tile([96, NB], bf16, tag=f"ARB{g}_{n}")
                        if n % 2 == 0:
                            nc.vector.tensor_copy(ar[:], sl)
                        else:
                            nc.scalar.copy(ar[:], sl)
                        ARBG[(g, n)] = ar

                # ---- 2 GRU passes, groups interleaved (3 independent chains) ----
                for pas in range(2):
                    for g in range(NGROUPS):
                        hg = HG[g]
                        ARB = [ARBG[(g, n)] for n in range(4)]
                        # Q_n = arb_n * h (both slots at once)
                        Q = []
                        for n in range(4):
                            q = sbp.tile([96, NB], bf16, tag=f"Q{n}")
                            if n >= 2:
                                nc.gpsimd.tensor_tensor(q[:], ARB[n][:], hg[0:96, :],
                                                        ALU.mult)
                            else:
                                nc.vector.tensor_mul(q[:], ARB[n][:], hg[0:96, :])
                            Q.append(q)
                        # hbar = sum_n SR_n @ Q_n
                        ps_hb = psp.tile([96, 2 * NB], f32, tag="ps_nh",
                                         name=f"pshb_{it}_{g}_{pas}")
                        for n in range(4):
                            nc.tensor.matmul(ps_hb[:, 0:NB], wt[f"SR{n}"][:], Q[n][:],
                                             start=(n == 0), stop=(n == 3))
                        hb = sbp.tile([96, NB], bf16, tag="HBs")
                        nc.vector.tensor_copy(hb[:], ps_hb[:, 0:NB])

                        # gates
                        ps_rz = psp.tile([96, 2 * NB], f32, tag="ps_rz",
                                         name=f"psrz_{it}_{g}_{pas}")
                        nc.tensor.matmul(ps_rz[:, 0:NB], wt["W_r_H"][:], hg[0:128, :],
                                         start=True, stop=False)
                        nc.tensor.matmul(ps_rz[:, 0:NB], wt["W_r_HB"][:], hb[:],
                                         start=False, stop=True)
                        nc.tensor.matmul(ps_rz[:, NB:2 * NB], wt["W_z_H"][:], hg[0:128, :],
                                         start=True, stop=False)
                        nc.tensor.matmul(ps_rz[:, NB:2 * NB], wt["W_z_HB"][:], hb[:],
                                         start=False, stop=True)
                        ps_nh = psp.tile([96, 2 * NB], f32, tag="ps_nh",
                                         name=f"psnh_{it}_{g}_{pas}")
                        nc.tensor.matmul(ps_nh[:, 0:NB], wt["W_inn_att"][96:128, :],
                                         hg[96:128, :], start=True, stop=False,
                                         tile_position=(96, 0))
                        nc.tensor.matmul(ps_nh[:, 0:NB], wt["W_inn_HB"][:], hb[:],
                                         start=False, stop=True)
                        nc.tensor.matmul(ps_nh[:, NB:2 * NB], wt["W_hn_H"][:],
                                         hg[0:96, :], start=True, stop=True)

                        srz = sbp.tile([96, 2 * NB], bf16, tag="SRZ")
                        nc.scalar.activation(srz[:, 0:NB], ps_rz[:, 0:NB], AF.Sigmoid,
                                             bias=wt["b_r96"][:, 0:1])
                        nc.scalar.activation(srz[:, NB:2 * NB], ps_rz[:, NB:2 * NB],
                                             AF.Sigmoid, bias=wt["b_z96"][:, 0:1])
                        t1 = sbp.tile([96, NB], f32, tag="t1")
                        nc.vector.scalar_tensor_tensor(t1[:], ps_nh[:, NB:2 * NB],
                                                       wt["b_hn96"][:, 0:1],
                                                       srz[:, 0:NB],
                                                       ALU.add, ALU.mult)
                        u = sbp.tile([96, NB], f32, tag="u")
                        nc.vector.scalar_tensor_tensor(u[:], ps_nh[:, 0:NB],
                                                       wt["b_inn96"][:, 0:1], t1[:],
                                                       ALU.add, ALU.add)
                        tn = sbp.tile([96, NB], bf16, tag="tn")
                        nc.scalar.activation(tn[:], u[:], AF.Tanh)
                        v = sbp.tile([96, NB], bf16, tag="v")
                        nc.vector.tensor_sub(v[:], hg[0:96, :], tn[:])
                        w2 = sbp.tile([96, NB], bf16, tag="w2")
                        nc.vector.tensor_mul(w2[:], srz[:, NB:2 * NB], v[:])
                        nc.vector.tensor_add(hg[0:96, :], tn[:], w2[:])

                # ---- BiLSTM over S steps (fw|bw paired along free dim) ----
                # XH_t [112, 2NB]: cols 0:NB fw, NB:2NB bw; rows 0:48 x, 48 ones
                # (bias row), 49:64 zero, 64:112 h. One tile per step, staged a
                # step ahead so the x DMAs are off the recurrence path.
                def stage_xh(t):
                    xh = lstmp.tile([112, 2 * NB], bf16, tag=f"XH{t % 2}",
                                    name=f"xh_{it}_{t}")
                    sft, sbt = t, 4 - t
                    nc.sync.dma_start(xh[48:64, :], wt["ones16"][:, :])
                    nc.sync.dma_start(
                        xh[0:48, 0:NB],
                        HG[sft // 2][(sft % 2) * 48:(sft % 2) * 48 + 48, :])
                    nc.sync.dma_start(
                        xh[0:48, NB:2 * NB],
                        HG[sbt // 2][(sbt % 2) * 48:(sbt % 2) * 48 + 48, :])
                    return xh

                C = lstmp.tile([48, 2 * NB], bf16, tag="C", name=f"c_{it}")
                HN = lstmp.tile([112, NB], bf16, tag="HN", name=f"hn_{it}")
                nc.vector.memset(C[:], 0.0)
                nc.vector.memset(HN[32:64, :], 0.0)
                XH = stage_xh(0)
                nc.vector.memset(XH[64:112, :], 0.0)
                for t in range(S):
                    ps_c1 = psp.tile([112, 2 * NB], f32, tag="ps_rz",
                                     name=f"psc1_{it}_{t}")
                    ps_c2 = psp.tile([112, 2 * NB], f32, tag="ps_nh",
                                     name=f"psc2_{it}_{t}")
                    for cname, ps in (("c1", ps_c1), ("c2", ps_c2)):
                        for hh, dirn in ((0, "fw"), (1, "bw")):
                            nc.tensor.matmul(ps[:, hh * NB:(hh + 1) * NB],
                                             wt[f"L_{cname}_{dirn}"][:],
                                             XH[:, hh * NB:(hh + 1) * NB],
                                             start=True, stop=True)
                    XHn = stage_xh(t + 1) if t < S - 1 else None
                    sio = sbp.tile([112, 2 * NB], bf16, tag="sio")
                    nc.scalar.activation(sio[:], ps_c1[:], AF.Sigmoid)
                    tg = sbp.tile([48, 2 * NB], bf16, tag="tg")
                    nc.scalar.activation(tg[:], ps_c2[0:48, :], AF.Tanh)
                    sf2 = sbp.tile([48, 2 * NB], bf16, tag="sf2")
                    nc.scalar.activation(sf2[:], ps_c2[64:112, :], AF.Sigmoid)
                    t1l = sbp.tile([48, 2 * NB], bf16, tag="t1l")
                    nc.vector.tensor_mul(t1l[:], sio[0:48, :], tg[:])
                    t2l = sbp.tile([48, 2 * NB], bf16, tag="t2l")
                    nc.vector.tensor_mul(t2l[:], sf2[:], C[:])
                    nc.vector.tensor_add(C[:], t1l[:], t2l[:])
                    tc2 = sbp.tile([112, 2 * NB], bf16, tag="tc2")
                    nc.scalar.activation(tc2[64:112, :], C[:], AF.Tanh)
                    if t < S - 1:
                        nc.vector.tensor_mul(XHn[64:112, :], sio[64:112, :],
                                             tc2[64:112, :])
                        XH = XHn
                    else:
                        nc.vector.tensor_mul(HN[0:48, :], sio[64:112, 0:NB],
                                             tc2[64:112, 0:NB])
                        nc.vector.tensor_mul(HN[64:112, :], sio[64:112, NB:2 * NB],
                                             tc2[64:112, NB:2 * NB])

                # ---- MLP ----
                psE = psp.tile([96, 2 * NB], f32, tag="ps_rz", name=f"psE_{it}")
                nc.tensor.matmul(psE[0:48, 0:NB], wt["We1"][:], HN[:],
                                 start=True, stop=True)
                e1 = sbp.tile([48, NB], bf16, tag="e1")
                nc.scalar.activation(e1[:], psE[0:48, 0:NB], AF.Relu,
                                     bias=wt["be1"][:, 0:1])
                psE2 = psp.tile([96, 2 * NB], f32, tag="ps_nh", name=f"psE2_{it}")
                nc.tensor.matmul(psE2[0:36, 0:NB], wt["We2"][:], e1[:],
                                 start=True, stop=True)
                e2 = sbp.tile([36, NB], bf16, tag="e2")
                nc.scalar.activation(e2[:], psE2[0:36, 0:NB], AF.Relu,
                                     bias=wt["be2"][:, 0:1])
                nc.tensor.matmul(psE[0:6, NB:2 * NB], wt["We3"][:], e2[:],
                                 start=True, stop=True)
                o = sbp.tile([6, NB], f32, tag="o")
                nc.scalar.activation(o[:], psE[0:6, NB:2 * NB], AF.Identity,
                                     bias=wt["be3"][:, 0:1])
                nc.sync.dma_start(out_d[:, c0:c0 + NB], o[:])

    split_excess_waits(nc)
    return nc


_NC_CACHE = None
TRACE = False
LAST_EXEC_NS = None


def kernel(nodes_feature, pos, attmat, **w):
    global _NC_CACHE, LAST_EXEC_NS
    from concourse.bass_utils import run_bass_kernel_spmd
    import concourse.mybir as mybir

    feat, att = prep_inputs(nodes_feature, pos, attmat)
    wts = build_weights(**w)

    if _NC_CACHE is None:
        _NC_CACHE = build_nc()
    nc = _NC_CACHE

    in_maps = []
    for c in range(NCORES):
        m = {"feat": np.ascontiguousarray(feat[:, :, c * NCORE:(c + 1) * NCORE]),
             "att": np.ascontiguousarray(att[:, :, c * NCORE:(c + 1) * NCORE])}
        for nm, shp, dt in WEIGHT_SPECS:
            m[nm] = wts[nm].astype(BF) if dt == mybir.dt.bfloat16 else wts[nm].astype(np.float32)
        in_maps.append(m)

    res = run_bass_kernel_spmd(nc, in_maps, core_ids=list(range(NCORES)),
                               trace=TRACE)
    LAST_EXEC_NS = res.exec_time_ns
    outs = [res.results[c]["out"] for c in range(NCORES)]     # [6, NCORE] each
    full = np.concatenate(outs, axis=1)                        # [6, N]
    return np.ascontiguousarray(full.T).astype(np.float32)     # [N, 6]


# revision 25
# speedup vs baseline: 1.1269x; 1.0462x over previous
"""Trainium2 Bass kernel for nn_Atomic_node_only_lstm (GNN message passing + BiLSTM + MLP).

v2: s-paired GRU with msg-linear folded into the gate matmuls.

Data-parallel over 8 NeuronCores (batch N=65536 -> 8192/core), NB=512 cols/tile.

Device layout (per batch tile of NB columns), seq steps grouped in pairs
(groups: (s0,s1), (s2,s3), (s4,zero)):
  H_g  [128, NB] bf16: rows 0:48 h_a, 48:96 h_b, 96:112 att_a, 112:128 att_b
       (h rows indexed k*12+d, k = node; att rows indexed n*4+w)
  arb_n [96, NB]: att[n,w] replicated over d, slot a rows 0:48, slot b 48:96
  Q_n  [96, NB] = arb_n * H_g[0:96]   (q_n[w*12+d] = att[n,w]*h[w,d], both slots)
  hbar [96, NB]: rows j*48+n*12+d = sum_w q over w   (slot j, node n)
  GRU gates from K=128 contraction of H_g (h part + att part folds s_n*(W_ih@msg_b))
  plus K=96 contraction of hbar (folds (W_ih@msg_w)).
  PSUM super-tiles [96, 1024] span 2 banks: rz = [r | z], nh = [inn | hn].

LSTM: XL [96, NB] = [x_fw | x_bw], HL [96, NB] = [h_fw | h_bw], C [96, NB].
  Per gate-type bank halves in free dim: B_if = [i | f], B_go = [g | o].
  x[j] = h[(k,d)], j = d*4+k (handled in lhsT construction).
MLP reads HL directly (en1_w cols 0:48 fw, 48:96 bw).
"""
import numpy as np
import ml_dtypes
from contextlib import ExitStack

N, S, K, D, H = 65536, 5, 4, 12, 48
NCORES = 8
NCORE = N // NCORES          # 8192 batch elements per core
NB = 512                     # batch columns per tile
NTILES = NCORE // NB
NGROUPS = 3                  # s-pairs: (0,1), (2,3), (4,-)

BF = ml_dtypes.bfloat16


# ----------------------------------------------------------------------------
# host-side weight construction (numpy, all tiny)
# ----------------------------------------------------------------------------
def build_weights(msg_w, msg_b, gru_w_ih, gru_w_hh, gru_b_ih, gru_b_hh,
                  lstm_w_ih_fw, lstm_w_hh_fw, lstm_b_ih_fw, lstm_b_hh_fw,
                  lstm_w_ih_bw, lstm_w_hh_bw, lstm_b_ih_bw, lstm_b_hh_bw,
                  en1_w, en1_b, en2_w, en2_b, en3_w, en3_b):
    out = {}
    # composed message->gate matrices [12 out-gate-d, 12 in-feat]
    A_r = gru_w_ih[0:12] @ msg_w      # gi_r = A_r @ hbar + s*(w_ih_r@msg_b)
    A_z = gru_w_ih[12:24] @ msg_w
    A_n = gru_w_ih[24:36] @ msg_w
    bi_r = gru_w_ih[0:12] @ msg_b     # [12]
    bi_z = gru_w_ih[12:24] @ msg_b
    bi_n = gru_w_ih[24:36] @ msg_b

    # RA_n lhsT [32, 96] (placed at partitions 96:128 on device; rhs = H_g[96:128]).
    # rows: 0:16 att_a (n*4+w), 16:32 att_b. cols: j*48 + w*12 + d.
    for n in range(4):
        R = np.zeros((32, 96), np.float32)
        for j in range(2):
            for w in range(4):
                for d in range(12):
                    R[j * 16 + n * 4 + w, j * 48 + w * 12 + d] = 1.0
        out[f"RA{n}"] = R

    # SR_n lhsT [96, 96]: contract Q_n -> hbar contribution of node n.
    # Q_n rows j*48 + w*12 + d ; hbar cols j*48 + n*12 + d.
    for n in range(4):
        Sm = np.zeros((96, 96), np.float32)
        for j in range(2):
            for w in range(4):
                for d in range(12):
                    Sm[j * 48 + w * 12 + d, j * 48 + n * 12 + d] = 1.0
        out[f"SR{n}"] = Sm

    # GRU gate lhsTs.
    # H-part [128, 96]: rhs = H_g (h rows j*48+k*12+dp, att rows 96+j*16+k*4+w)
    # out cols j*48 + k*12 + d.
    def gate_H(whh_blk, bi_blk):
        Wm = np.zeros((128, 96), np.float32)
        for j in range(2):
            for k in range(4):
                for d in range(12):
                    c = j * 48 + k * 12 + d
                    if whh_blk is not None:
                        for dp in range(12):
                            Wm[j * 48 + k * 12 + dp, c] = whh_blk[d, dp]
                    if bi_blk is not None:
                        for w in range(4):
                            Wm[96 + j * 16 + k * 4 + w, c] = bi_blk[d]
        return Wm

    def gate_HB(A_blk):
        Wm = np.zeros((96, 96), np.float32)
        for j in range(2):
            for k in range(4):
                for d in range(12):
                    c = j * 48 + k * 12 + d
                    for dp in range(12):
                        Wm[j * 48 + k * 12 + dp, c] = A_blk[d, dp]
        return Wm

    out["W_r_H"] = gate_H(gru_w_hh[0:12], bi_r)
    out["W_z_H"] = gate_H(gru_w_hh[12:24], bi_z)
    out["W_hn_H"] = gate_H(gru_w_hh[24:36], None)[0:96]   # h rows only, K=96
    out["W_inn_att"] = gate_H(None, bi_n)[96:128]         # att rows only, K=32
    out["W_r_HB"] = gate_HB(A_r)
    out["W_z_HB"] = gate_HB(A_z)
    out["W_inn_HB"] = gate_HB(A_n)

    out["b_r96"] = np.tile(gru_b_ih[0:12] + gru_b_hh[0:12], 8).reshape(96, 1).astype(np.float32)
    out["b_z96"] = np.tile(gru_b_ih[12:24] + gru_b_hh[12:24], 8).reshape(96, 1).astype(np.float32)
    out["b_inn96"] = np.tile(gru_b_ih[24:36], 8).reshape(96, 1).astype(np.float32)
    out["b_hn96"] = np.tile(gru_b_hh[24:36], 8).reshape(96, 1).astype(np.float32)
    # rz super-bias [96,1] applies to both halves? No - bias is per partition, free
    # dim halves share it. r and z biases differ -> separate ACT calls? No:
    # sigma(rz super-tile) is ONE act over [96, 1024]; bias per-partition only.
    # So we must fold r/z biases into the matmuls instead: add via att-part?
    # Simpler: bias rows are per-partition; r occupies cols 0:512 and z 512:1024 of
    # the SAME partitions -> per-partition bias cannot differ. Fold biases into
    # lhsT via the constant-1 trick: use att rows? att sums vary. Instead keep
    # two ACT calls when biases differ. For rz we instead ADD the bias inside the
    # H-part lhsT using a constant row... no constant row exists in H_g.
    # Resolution: biases b_r/b_z are added via activation bias -> need separate
    # sigma(r), sigma(z). To keep ONE act per super-tile we instead pre-add the
    # bias by augmenting att columns: s_n terms already use att rows; a constant
    # offset cannot come from data. So: two ACTs for rz after all (see kernel).

    # LSTM lhsTs.
    # XL-part [96, 96]: rhs = XL = [x_fw | x_bw]; rows j*48 + k*12 + d (x of dir j)
    # cols j*48 + m. x[jj] = h[(k,d)], jj = d*4+k -> weight w_ih[g*48+m, d*4+k].
    # HL-part [96, 96]: rhs = HL = [h_fw | h_bw]; rows j*48 + jj; cols j*48 + m.
    gates = {"I": 0, "F": 1, "G": 2, "O": 3}
    wih = {"fw": lstm_w_ih_fw, "bw": lstm_w_ih_bw}
    whh = {"fw": lstm_w_hh_fw, "bw": lstm_w_hh_bw}
    bih = {"fw": lstm_b_ih_fw, "bw": lstm_b_ih_bw}
    bhh = {"fw": lstm_b_hh_fw, "bw": lstm_b_hh_bw}
    for gname, g in gates.items():
        X = np.zeros((96, 96), np.float32)
        Hm = np.zeros((96, 96), np.float32)
        b = np.zeros((96, 1), np.float32)
        for j, dirn in enumerate(("fw", "bw")):
            wg = wih[dirn][g * 48:(g + 1) * 48, :]   # [48 out m, 48 xin jj]
            hg = whh[dirn][g * 48:(g + 1) * 48, :]
            for k in range(4):
                for d in range(12):
                    X[j * 48 + k * 12 + d, j * 48:(j + 1) * 48] = wg[:, d * 4 + k]
            Hm[j * 48:(j + 1) * 48, j * 48:(j + 1) * 48] = hg.T
            b[j * 48:(j + 1) * 48, 0] = (bih[dirn][g * 48:(g + 1) * 48]
                                         + bhh[dirn][g * 48:(g + 1) * 48])
        out[f"L_{gname}_X"] = X
        out[f"L_{gname}_HL"] = Hm
        out[f"b_{gname}96"] = b

    # MLP: HL = [h_fw(0:48) | h_bw(48:96)]
    We1 = np.zeros((96, 48), np.float32)
    We1[0:48, :] = en1_w[:, 0:48].T
    We1[48:96, :] = en1_w[:, 48:96].T
    out["We1"] = We1
    out["be1"] = en1_b.reshape(48, 1).astype(np.float32)
    out["We2"] = en2_w.T.copy()                 # [48, 36]
    out["be2"] = en2_b.reshape(36, 1).astype(np.float32)
    out["We3"] = en3_w.T.copy()                 # [36, 6]
    out["be3"] = en3_b.reshape(6, 1).astype(np.float32)
    return out


def prep_inputs(nodes_feature, pos, attmat):
    """Full-batch host layout: feat [S, 48, N] bf16 (k-major (k,d)), att [S, 16, N] bf16."""
    nf = np.concatenate([nodes_feature, pos], axis=-1)       # [N,S,K,12]
    feat = np.ascontiguousarray(nf.transpose(1, 2, 3, 0)).reshape(S, 48, N)
    att = np.ascontiguousarray(attmat.transpose(1, 2, 3, 0)).reshape(S, 16, N)
    return feat.astype(BF), att.astype(BF)


# ----------------------------------------------------------------------------
# device kernel builder
# ----------------------------------------------------------------------------
def split_excess_waits(nc, max_waits=1):
    import concourse.mybir as mybir
    cnt = 0
    for f in nc.m.functions:
        for bb in f.blocks:
            insts = bb.instructions
            new = []
            changed = False
            for inst in insts:
                si = inst.sync_info
                waits = list(si.on_wait) if si and si.on_wait else []
                if len(waits) > max_waits:
                    changed = True
                    k = 0
                    while len(waits) - k > max_waits:
                        chunk = waits[k:k + max_waits]
                        k += max_waits
                        cnt += 1
                        nop = mybir.InstNoOp(name=f"waitsplit-{cnt}", ins=[], outs=[])
                        nop.engine = inst.engine
                        nop.sync_info = mybir.SyncInfo(on_wait=chunk, on_update=[])
                        new.append(nop)
                    inst.sync_info = mybir.SyncInfo(
                        on_wait=waits[k:],
                        on_update=list(si.on_update) if si.on_update else [])
                new.append(inst)
            if changed:
                bb.instructions = new
    return cnt


WEIGHT_SPECS = None  # filled in build_nc


def build_nc():
    import concourse.bass as bass
    import concourse.tile as tile
    from concourse import mybir

    f32 = mybir.dt.float32
    bf16 = mybir.dt.bfloat16
    AF = mybir.ActivationFunctionType
    ALU = mybir.AluOpType

    nc = bass.Bass("TRN2")

    feat_d = nc.dram_tensor("feat", [S, 48, NCORE], bf16, kind="ExternalInput")
    att_d = nc.dram_tensor("att", [S, 16, NCORE], bf16, kind="ExternalInput")

    wspecs = []
    for n in range(4):
        wspecs.append((f"RA{n}", (32, 96), bf16))
        wspecs.append((f"SR{n}", (96, 96), bf16))
    wspecs += [("W_r_H", (128, 96), bf16), ("W_z_H", (128, 96), bf16),
               ("W_hn_H", (96, 96), bf16), ("W_inn_att", (32, 96), bf16),
               ("W_r_HB", (96, 96), bf16), ("W_z_HB", (96, 96), bf16),
               ("W_inn_HB", (96, 96), bf16),
               ("b_r96", (96, 1), f32), ("b_z96", (96, 1), f32),
               ("b_inn96", (96, 1), f32), ("b_hn96", (96, 1), f32)]
    for gname in "IFGO":
        wspecs.append((f"L_{gname}_X", (96, 96), bf16))
        wspecs.append((f"L_{gname}_HL", (96, 96), bf16))
        wspecs.append((f"b_{gname}96", (96, 1), f32))
    wspecs += [("We1", (96, 48), bf16), ("be1", (48, 1), f32),
               ("We2", (48, 36), bf16), ("be2", (36, 1), f32),
               ("We3", (36, 6), bf16), ("be3", (6, 1), f32)]

    wnames = {}
    for nm, shp, dt in wspecs:
        wnames[nm] = nc.dram_tensor(nm, list(shp), dt, kind="ExternalInput")
    out_d = nc.dram_tensor("out", [6, NCORE], f32, kind="ExternalOutput")
    RA_QUAD = {0: 0, 1: 32, 2: 64, 3: 96}  # row-quadrant per RA matmul

    global WEIGHT_SPECS
    WEIGHT_SPECS = wspecs

    with tile.TileContext(nc) as tc:
        with ExitStack() as ctx:
            wpool = ctx.enter_context(tc.tile_pool(name="weights", bufs=1))
            wt = {}
            for nm, shp, dt in wspecs:
                # lhsT base partition must match the rhs row quadrant:
                # RA{n} runs at row quadrant 32n; W_inn_att at quadrant 96.
                if nm.startswith("RA"):
                    q = RA_QUAD[int(nm[2])]
                    t = wpool.tile([128, shp[1]], dt, tag=f"w_{nm}")
                    nc.sync.dma_start(t[q:q + 32, :], wnames[nm][:])
                    wt[nm] = t
                elif nm == "W_inn_att":
                    t = wpool.tile([128, shp[1]], dt, tag=f"w_{nm}")
                    nc.sync.dma_start(t[96:128, :], wnames[nm][:])
                    wt[nm] = t
                else:
                    t = wpool.tile([shp[0], shp[1]], dt, tag=f"w_{nm}")
                    nc.sync.dma_start(t[:], wnames[nm][:])
                    wt[nm] = t

            hpool = ctx.enter_context(tc.tile_pool(name="hg", bufs=2))
            arpool = ctx.enter_context(tc.tile_pool(name="ar", bufs=2))
            sbp = ctx.enter_context(tc.tile_pool(name="work", bufs=3))
            lstmp = ctx.enter_context(tc.tile_pool(name="lstm", bufs=2))
            psp = ctx.enter_context(tc.tile_pool(name="ps", bufs=2, space="PSUM"))

            for it in range(NTILES):
                c0 = it * NB
                # ---- load groups ----
                HG = []
                for g in range(NGROUPS):
                    hg = hpool.tile([128, NB], bf16, tag=f"HG{g}")
                    sa = 2 * g
                    if g == 2:
                        # zero slot-b rows first (partition offsets must be 32-aligned;
                        # DMAs below restore the overlapping real rows)
                        nc.vector.memset(hg[32:64, :], 0.0)
                        nc.vector.memset(hg[64:96, :], 0.0)
                        nc.vector.memset(hg[96:128, :], 0.0)
                    nc.sync.dma_start(hg[0:48, :], feat_d[sa, :, c0:c0 + NB])
                    nc.sync.dma_start(hg[96:112, :], att_d[sa, :, c0:c0 + NB])
                    if g < 2:
                        sb = 2 * g + 1
                        nc.sync.dma_start(hg[48:96, :], feat_d[sb, :, c0:c0 + NB])
                        nc.sync.dma_start(hg[112:128, :], att_d[sb, :, c0:c0 + NB])
                    HG.append(hg)

                # att replicated into all 4 row-quadrants so the 4 RA matmuls can
                # run concurrently in distinct PE row groups (quadrant 3 = hg).
                AT4 = []
                for g in range(NGROUPS):
                    at4 = hpool.tile([96, NB], bf16, tag=f"AT4{g}")
                    if g == 2:
                        nc.vector.memset(at4[:], 0.0)
                    for q in range(3):
                        nc.sync.dma_start(at4[32 * q:32 * q + 16, :],
                                          att_d[2 * g, :, c0:c0 + NB])
                        if g < 2:
                            nc.sync.dma_start(at4[32 * q + 16:32 * q + 32, :],
                                              att_d[2 * g + 1, :, c0:c0 + NB])
                    AT4.append(at4)

                # ---- attention replication for all groups ----
                ARBG = {}
                for g in range(NGROUPS):
                    hg = HG[g]
                    ps_a = psp.tile([96, 2 * NB], f32, tag="ps_rz", name=f"psa_{it}_{g}")
                    ps_b = psp.tile([96, 2 * NB], f32, tag="ps_nh", name=f"psb_{it}_{g}")
                    for n in range(4):
                        ps = (ps_a, ps_b)[n // 2]
                        sl = ps[:, (n % 2) * NB:(n % 2 + 1) * NB]
                        q = RA_QUAD[n]
                        rhs = (hg[96:128, :] if n == 3
                               else AT4[g][q:q + 32, :])
                        nc.tensor.matmul(sl, wt[f"RA{n}"][q:q + 32, :], rhs,
                                         start=True, stop=True,
                                         tile_position=(q, 0))
                        ar = arpool.tile([96, NB], bf16, tag=f"ARB{g}_{n}")
                        if n % 2 == 0:
                            nc.vector.tensor_copy(ar[:], sl)
                        else:
                            nc.scalar.copy(ar[:], sl)
                        ARBG[(g, n)] = ar

                # ---- 2 GRU passes, groups interleaved (3 independent chains) ----
                for pas in range(2):
                    for g in range(NGROUPS):
                        hg = HG[g]
                        ARB = [ARBG[(g, n)] for n in range(4)]
                        # Q_n = arb_n * h (both slots at once)
                        Q = []
                        for n in range(4):
                            q = sbp.tile([96, NB], bf16, tag=f"Q{n}")
                            if n >= 2:
                                nc.gpsimd.tensor_tensor(q[:], ARB[n][:], hg[0:96, :],
                                                        ALU.mult)
                            else:
                                nc.vector.tensor_mul(q[:], ARB[n][:], hg[0:96, :])
                            Q.append(q)
                        # hbar = sum_n SR_n @ Q_n
                        ps_hb = psp.tile([96, 2 * NB], f32, tag="ps_nh",
                                         name=f"pshb_{it}_{g}_{pas}")
                        for n in range(4):
                            nc.tensor.matmul(ps_hb[:, 0:NB], wt[f"SR{n}"][:], Q[n][:],
                                             start=(n == 0), stop=(n == 3))
                        hb = sbp.tile([96, NB], bf16, tag="HBs")
                        nc.vector.tensor_copy(hb[:], ps_hb[:, 0:NB])

                        # gates
                        ps_rz = psp.tile([96, 2 * NB], f32, tag="ps_rz",
                                         name=f"psrz_{it}_{g}_{pas}")
                        nc.tensor.matmul(ps_rz[:, 0:NB], wt["W_r_H"][:], hg[0:128, :],
                                         start=True, stop=False)
                        nc.tensor.matmul(ps_rz[:, 0:NB], wt["W_r_HB"][:], hb[:],
                                         start=False, stop=True)
                        nc.tensor.matmul(ps_rz[:, NB:2 * NB], wt["W_z_H"][:], hg[0:128, :],
                                         start=True, stop=False)
                        nc.tensor.matmul(ps_rz[:, NB:2 * NB], wt["W_z_HB"][:], hb[:],
                                         start=False, stop=True)
                        ps_nh = psp.tile([96, 2 * NB], f32, tag="ps_nh",
                                         name=f"psnh_{it}_{g}_{pas}")
                        nc.tensor.matmul(ps_nh[:, 0:NB], wt["W_inn_att"][96:128, :],
                                         hg[96:128, :], start=True, stop=False,
                                         tile_position=(96, 0))
                        nc.tensor.matmul(ps_nh[:, 0:NB], wt["W_inn_HB"][:], hb[:],
                                         start=False, stop=True)
                        nc.tensor.matmul(ps_nh[:, NB:2 * NB], wt["W_hn_H"][:],
                                         hg[0:96, :], start=True, stop=True)

                        srz = sbp.tile([96, 2 * NB], bf16, tag="SRZ")
                        nc.scalar.activation(srz[:, 0:NB], ps_rz[:, 0:NB], AF.Sigmoid,
                                             bias=wt["b_r96"][:, 0:1])
                        nc.scalar.activation(srz[:, NB:2 * NB], ps_rz[:, NB:2 * NB],
                                             AF.Sigmoid, bias=wt["b_z96"][:, 0:1])
                        t1 = sbp.tile([96, NB], f32, tag="t1")
                        nc.vector.scalar_tensor_tensor(t1[:], ps_nh[:, NB:2 * NB],
                                                       wt["b_hn96"][:, 0:1],
                                                       srz[:, 0:NB],
                                                       ALU.add, ALU.mult)
                        u = sbp.tile([96, NB], f32, tag="u")
                        nc.vector.scalar_tensor_tensor(u[:], ps_nh[:, 0:NB],
                                                       wt["b_inn96"][:, 0:1], t1[:],
                                                       ALU.add, ALU.add)
                        tn = sbp.tile([96, NB], bf16, tag="tn")
                        nc.scalar.activation(tn[:], u[:], AF.Tanh)
                        v = sbp.tile([96, NB], bf16, tag="v")
                        nc.vector.tensor_sub(v[:], hg[0:96, :], tn[:])
                        w2 = sbp.tile([96, NB], bf16, tag="w2")
                        nc.vector.tensor_mul(w2[:], srz[:, NB:2 * NB], v[:])
                        nc.vector.tensor_add(hg[0:96, :], tn[:], w2[:])

                # ---- BiLSTM over S steps ----
                HL = lstmp.tile([96, NB], bf16, tag="HL", name=f"hl_{it}")
                C = lstmp.tile([96, NB], bf16, tag="C", name=f"c_{it}")
                nc.vector.memset(HL[:], 0.0)
                nc.vector.memset(C[:], 0.0)
                for t in range(S):
                    sf_, sb_ = t, 4 - t
                    xl = lstmp.tile([96, NB], bf16, tag="XL")
                    nc.sync.dma_start(
                        xl[0:48, :], HG[sf_ // 2][(sf_ % 2) * 48:(sf_ % 2) * 48 + 48, :])
                    nc.sync.dma_start(
                        xl[48:96, :], HG[sb_ // 2][(sb_ % 2) * 48:(sb_ % 2) * 48 + 48, :])
                    ps_if = psp.tile([96, 2 * NB], f32, tag="ps_rz",
                                     name=f"psif_{it}_{t}")
                    ps_go = psp.tile([96, 2 * NB], f32, tag="ps_nh",
                                     name=f"psgo_{it}_{t}")
                    for gname, ps, half in (("I", ps_if, 0), ("F", ps_if, 1),
                                            ("G", ps_go, 0), ("O", ps_go, 1)):
                        sl = ps[:, half * NB:(half + 1) * NB]
                        nc.tensor.matmul(sl, wt[f"L_{gname}_X"][:], xl[:],
                                         start=True, stop=False)
                        nc.tensor.matmul(sl, wt[f"L_{gname}_HL"][:], HL[:],
                                         start=False, stop=True)
                    si = sbp.tile([96, NB], bf16, tag="si")
                    nc.scalar.activation(si[:], ps_if[:, 0:NB], AF.Sigmoid,
                                         bias=wt["b_I96"][:, 0:1])
                    sf2 = sbp.tile([96, NB], bf16, tag="sf2")
                    nc.scalar.activation(sf2[:], ps_if[:, NB:2 * NB], AF.Sigmoid,
                                         bias=wt["b_F96"][:, 0:1])
                    tg = sbp.tile([96, NB], bf16, tag="tg")
                    nc.scalar.activation(tg[:], ps_go[:, 0:NB], AF.Tanh,
                                         bias=wt["b_G96"][:, 0:1])
                    t1l = sbp.tile([96, NB], bf16, tag="t1l")
                    nc.vector.tensor_mul(t1l[:], si[:], tg[:])
                    t2l = sbp.tile([96, NB], bf16, tag="t2l")
                    nc.gpsimd.tensor_mul(t2l[:], sf2[:], C[:])
                    nc.vector.tensor_add(C[:], t1l[:], t2l[:])
                    tc2 = sbp.tile([96, NB], bf16, tag="tc2")
                    nc.scalar.activation(tc2[:], C[:], AF.Tanh)
                    so = sbp.tile([96, NB], bf16, tag="so")
                    nc.scalar.activation(so[:], ps_go[:, NB:2 * NB], AF.Sigmoid,
                                         bias=wt["b_O96"][:, 0:1])
                    nc.vector.tensor_mul(HL[:], so[:], tc2[:])

                # ---- MLP ----
                psE = psp.tile([96, 2 * NB], f32, tag="ps_rz", name=f"psE_{it}")
                nc.tensor.matmul(psE[0:48, 0:NB], wt["We1"][:], HL[:],
                                 start=True, stop=True)
                e1 = sbp.tile([48, NB], bf16, tag="e1")
                nc.scalar.activation(e1[:], psE[0:48, 0:NB], AF.Relu,
                                     bias=wt["be1"][:, 0:1])
                psE2 = psp.tile([96, 2 * NB], f32, tag="ps_nh", name=f"psE2_{it}")
                nc.tensor.matmul(psE2[0:36, 0:NB], wt["We2"][:], e1[:],
                                 start=True, stop=True)
                e2 = sbp.tile([36, NB], bf16, tag="e2")
                nc.scalar.activation(e2[:], psE2[0:36, 0:NB], AF.Relu,
                                     bias=wt["be2"][:, 0:1])
                nc.tensor.matmul(psE[0:6, NB:2 * NB], wt["We3"][:], e2[:],
                                 start=True, stop=True)
                o = sbp.tile([6, NB], f32, tag="o")
                nc.scalar.activation(o[:], psE[0:6, NB:2 * NB], AF.Identity,
                                     bias=wt["be3"][:, 0:1])
                nc.sync.dma_start(out_d[:, c0:c0 + NB], o[:])

    split_excess_waits(nc)
    return nc


_NC_CACHE = None
TRACE = False
LAST_EXEC_NS = None


def kernel(nodes_feature, pos, attmat, **w):
    global _NC_CACHE, LAST_EXEC_NS
    from concourse.bass_utils import run_bass_kernel_spmd
    import concourse.mybir as mybir

    feat, att = prep_inputs(nodes_feature, pos, attmat)
    wts = build_weights(**w)

    if _NC_CACHE is None:
        _NC_CACHE = build_nc()
    nc = _NC_CACHE

    in_maps = []
    for c in range(NCORES):
        m = {"feat": np.ascontiguousarray(feat[:, :, c * NCORE:(c + 1) * NCORE]),
             "att": np.ascontiguousarray(att[:, :, c * NCORE:(c + 1) * NCORE])}
        for nm, shp, dt in WEIGHT_SPECS:
            m[nm] = wts[nm].astype(BF) if dt == mybir.dt.bfloat16 else wts[nm].astype(np.float32)
        in_maps.append(m)

    res = run_bass_kernel_spmd(nc, in_maps, core_ids=list(range(NCORES)),
                               trace=TRACE)
    LAST_EXEC_NS = res.exec_time_ns
    outs = [res.results[c]["out"] for c in range(NCORES)]     # [6, NCORE] each
    full = np.concatenate(outs, axis=1)                        # [6, N]
    return np.ascontiguousarray(full.T).astype(np.float32)     # [N, 6]


# revision 31
# speedup vs baseline: 1.5177x; 1.3469x over previous
"""Trainium2 Bass kernel for nn_Atomic_node_only_lstm (GNN message passing + BiLSTM + MLP).

v2: s-paired GRU with msg-linear folded into the gate matmuls.

Data-parallel over 8 NeuronCores (batch N=65536 -> 8192/core), NB=512 cols/tile.

Device layout (per batch tile of NB columns), seq steps grouped in pairs
(groups: (s0,s1), (s2,s3), (s4,zero)):
  H_g  [128, NB] bf16: rows 0:48 h_a, 48:96 h_b, 96:112 att_a, 112:128 att_b
       (h rows indexed k*12+d, k = node; att rows indexed n*4+w)
  arb_n [96, NB]: att[n,w] replicated over d, slot a rows 0:48, slot b 48:96
  Q_n  [96, NB] = arb_n * H_g[0:96]   (q_n[w*12+d] = att[n,w]*h[w,d], both slots)
  hbar [96, NB]: rows j*48+n*12+d = sum_w q over w   (slot j, node n)
  GRU gates from K=128 contraction of H_g (h part + att part folds s_n*(W_ih@msg_b))
  plus K=96 contraction of hbar (folds (W_ih@msg_w)).
  PSUM super-tiles [96, 1024] span 2 banks: rz = [r | z], nh = [inn | hn].

LSTM: XL [96, NB] = [x_fw | x_bw], HL [96, NB] = [h_fw | h_bw], C [96, NB].
  Per gate-type bank halves in free dim: B_if = [i | f], B_go = [g | o].
  x[j] = h[(k,d)], j = d*4+k (handled in lhsT construction).
MLP reads HL directly (en1_w cols 0:48 fw, 48:96 bw).
"""
import numpy as np
import ml_dtypes
from contextlib import ExitStack

N, S, K, D, H = 65536, 5, 4, 12, 48
NCORES = 8
NCORE = N // NCORES          # 8192 batch elements per core
NB = 512                     # batch columns per tile
NTILES = NCORE // NB
NGROUPS = 3                  # s-pairs: (0,1), (2,3), (4,-)

BF = ml_dtypes.bfloat16


# ----------------------------------------------------------------------------
# host-side weight construction (numpy, all tiny)
# ----------------------------------------------------------------------------
def build_weights(msg_w, msg_b, gru_w_ih, gru_w_hh, gru_b_ih, gru_b_hh,
                  lstm_w_ih_fw, lstm_w_hh_fw, lstm_b_ih_fw, lstm_b_hh_fw,
                  lstm_w_ih_bw, lstm_w_hh_bw, lstm_b_ih_bw, lstm_b_hh_bw,
                  en1_w, en1_b, en2_w, en2_b, en3_w, en3_b):
    out = {}
    # composed message->gate matrices [12 out-gate-d, 12 in-feat]
    A_r = gru_w_ih[0:12] @ msg_w      # gi_r = A_r @ hbar + s*(w_ih_r@msg_b)
    A_z = gru_w_ih[12:24] @ msg_w
    A_n = gru_w_ih[24:36] @ msg_w
    bi_r = gru_w_ih[0:12] @ msg_b     # [12]
    bi_z = gru_w_ih[12:24] @ msg_b
    bi_n = gru_w_ih[24:36] @ msg_b

    # RA_n lhsT [32, 96] (placed at partitions 96:128 on device; rhs = H_g[96:128]).
    # rows: 0:16 att_a (n*4+w), 16:32 att_b. cols: j*48 + w*12 + d.
    for n in range(4):
        R = np.zeros((32, 96), np.float32)
        for j in range(2):
            for w in range(4):
                for d in range(12):
                    R[j * 16 + n * 4 + w, j * 48 + w * 12 + d] = 1.0
        out[f"RA{n}"] = R

    # SR_n lhsT [96, 96]: contract Q_n -> hbar contribution of node n.
    # Q_n rows j*48 + w*12 + d ; hbar cols j*48 + n*12 + d.
    for n in range(4):
        Sm = np.zeros((96, 96), np.float32)
        for j in range(2):
            for w in range(4):
                for d in range(12):
                    Sm[j * 48 + w * 12 + d, j * 48 + n * 12 + d] = 1.0
        out[f"SR{n}"] = Sm

    # GRU gate lhsTs.
    # H-part [128, 96]: rhs = H_g (h rows j*48+k*12+dp, att rows 96+j*16+k*4+w)
    # out cols j*48 + k*12 + d.
    def gate_H(whh_blk, bi_blk):
        Wm = np.zeros((128, 96), np.float32)
        for j in range(2):
            for k in range(4):
                for d in range(12):
                    c = j * 48 + k * 12 + d
                    if whh_blk is not None:
                        for dp in range(12):
                            Wm[j * 48 + k * 12 + dp, c] = whh_blk[d, dp]
                    if bi_blk is not None:
                        for w in range(4):
                            Wm[96 + j * 16 + k * 4 + w, c] = bi_blk[d]
        return Wm

    def gate_HB(A_blk):
        Wm = np.zeros((96, 96), np.float32)
        for j in range(2):
            for k in range(4):
                for d in range(12):
                    c = j * 48 + k * 12 + d
                    for dp in range(12):
                        Wm[j * 48 + k * 12 + dp, c] = A_blk[d, dp]
        return Wm

    out["W_r_H"] = gate_H(gru_w_hh[0:12], bi_r)
    out["W_z_H"] = gate_H(gru_w_hh[12:24], bi_z)
    out["W_hn_H"] = gate_H(gru_w_hh[24:36], None)[0:96]   # h rows only, K=96
    out["W_inn_att"] = gate_H(None, bi_n)[96:128]         # att rows only, K=32
    out["W_r_HB"] = gate_HB(A_r)
    out["W_z_HB"] = gate_HB(A_z)
    out["W_inn_HB"] = gate_HB(A_n)

    out["b_r96"] = np.tile(gru_b_ih[0:12] + gru_b_hh[0:12], 8).reshape(96, 1).astype(np.float32)
    out["b_z96"] = np.tile(gru_b_ih[12:24] + gru_b_hh[12:24], 8).reshape(96, 1).astype(np.float32)
    out["b_inn96"] = np.tile(gru_b_ih[24:36], 8).reshape(96, 1).astype(np.float32)
    out["b_hn96"] = np.tile(gru_b_hh[24:36], 8).reshape(96, 1).astype(np.float32)
    # rz super-bias [96,1] applies to both halves? No - bias is per partition, free
    # dim halves share it. r and z biases differ -> separate ACT calls? No:
    # sigma(rz super-tile) is ONE act over [96, 1024]; bias per-partition only.
    # So we must fold r/z biases into the matmuls instead: add via att-part?
    # Simpler: bias rows are per-partition; r occupies cols 0:512 and z 512:1024 of
    # the SAME partitions -> per-partition bias cannot differ. Fold biases into
    # lhsT via the constant-1 trick: use att rows? att sums vary. Instead keep
    # two ACT calls when biases differ. For rz we instead ADD the bias inside the
    # H-part lhsT using a constant row... no constant row exists in H_g.
    # Resolution: biases b_r/b_z are added via activation bias -> need separate
    # sigma(r), sigma(z). To keep ONE act per super-tile we instead pre-add the
    # bias by augmenting att columns: s_n terms already use att rows; a constant
    # offset cannot come from data. So: two ACTs for rz after all (see kernel).

    # LSTM lhsTs.
    # XL-part [96, 96]: rhs = XL = [x_fw | x_bw]; rows j*48 + k*12 + d (x of dir j)
    # cols j*48 + m. x[jj] = h[(k,d)], jj = d*4+k -> weight w_ih[g*48+m, d*4+k].
    # HL-part [96, 96]: rhs = HL = [h_fw | h_bw]; rows j*48 + jj; cols j*48 + m.
    gates = {"I": 0, "F": 1, "G": 2, "O": 3}
    wih = {"fw": lstm_w_ih_fw, "bw": lstm_w_ih_bw}
    whh = {"fw": lstm_w_hh_fw, "bw": lstm_w_hh_bw}
    bih = {"fw": lstm_b_ih_fw, "bw": lstm_b_ih_bw}
    bhh = {"fw": lstm_b_hh_fw, "bw": lstm_b_hh_bw}
    for gname, g in gates.items():
        X = np.zeros((96, 96), np.float32)
        Hm = np.zeros((96, 96), np.float32)
        b = np.zeros((96, 1), np.float32)
        for j, dirn in enumerate(("fw", "bw")):
            wg = wih[dirn][g * 48:(g + 1) * 48, :]   # [48 out m, 48 xin jj]
            hg = whh[dirn][g * 48:(g + 1) * 48, :]
            for k in range(4):
                for d in range(12):
                    X[j * 48 + k * 12 + d, j * 48:(j + 1) * 48] = wg[:, d * 4 + k]
            Hm[j * 48:(j + 1) * 48, j * 48:(j + 1) * 48] = hg.T
            b[j * 48:(j + 1) * 48, 0] = (bih[dirn][g * 48:(g + 1) * 48]
                                         + bhh[dirn][g * 48:(g + 1) * 48])
        out[f"L_{gname}_X"] = X
        out[f"L_{gname}_HL"] = Hm
        out[f"b_{gname}96"] = b

    # MLP: HL = [h_fw(0:48) | h_bw(48:96)]
    We1 = np.zeros((96, 48), np.float32)
    We1[0:48, :] = en1_w[:, 0:48].T
    We1[48:96, :] = en1_w[:, 48:96].T
    out["We1"] = We1
    out["be1"] = en1_b.reshape(48, 1).astype(np.float32)
    out["We2"] = en2_w.T.copy()                 # [48, 36]
    out["be2"] = en2_b.reshape(36, 1).astype(np.float32)
    out["We3"] = en3_w.T.copy()                 # [36, 6]
    out["be3"] = en3_b.reshape(6, 1).astype(np.float32)
    return out


def prep_inputs(nodes_feature, pos, attmat):
    """Full-batch host layout: feat [S, 48, N] bf16 (k-major (k,d)), att [S, 16, N] bf16."""
    nf = np.concatenate([nodes_feature, pos], axis=-1)       # [N,S,K,12]
    feat = np.ascontiguousarray(nf.transpose(1, 2, 3, 0)).reshape(S, 48, N)
    att = np.ascontiguousarray(attmat.transpose(1, 2, 3, 0)).reshape(S, 16, N)
    return feat.astype(BF), att.astype(BF)


# ----------------------------------------------------------------------------
# device kernel builder
# ----------------------------------------------------------------------------
def split_excess_waits(nc, max_waits=1):
    import concourse.mybir as mybir
    cnt = 0
    for f in nc.m.functions:
        for bb in f.blocks:
            insts = bb.instructions
            new = []
            changed = False
            for inst in insts:
                si = inst.sync_info
                waits = list(si.on_wait) if si and si.on_wait else []
                if len(waits) > max_waits:
                    changed = True
                    k = 0
                    while len(waits) - k > max_waits:
                        chunk = waits[k:k + max_waits]
                        k += max_waits
                        cnt += 1
                        nop = mybir.InstNoOp(name=f"waitsplit-{cnt}", ins=[], outs=[])
                        nop.engine = inst.engine
                        nop.sync_info = mybir.SyncInfo(on_wait=chunk, on_update=[])
                        new.append(nop)
                    inst.sync_info = mybir.SyncInfo(
                        on_wait=waits[k:],
                        on_update=list(si.on_update) if si.on_update else [])
                new.append(inst)
            if changed:
                bb.instructions = new
    return cnt


WEIGHT_SPECS = None  # filled in build_nc


def build_nc():
    import concourse.bass as bass
    import concourse.tile as tile
    from concourse import mybir

    f32 = mybir.dt.float32
    bf16 = mybir.dt.bfloat16
    AF = mybir.ActivationFunctionType
    ALU = mybir.AluOpType

    nc = bass.Bass("TRN2")

    feat_d = nc.dram_tensor("feat", [S, 48, NCORE], bf16, kind="ExternalInput")
    att_d = nc.dram_tensor("att", [S, 16, NCORE], bf16, kind="ExternalInput")

    wspecs = []
    for n in range(4):
        wspecs.append((f"RA{n}", (32, 96), bf16))
        wspecs.append((f"SR{n}", (96, 96), bf16))
    wspecs += [("W_r_H", (128, 96), bf16), ("W_z_H", (128, 96), bf16),
               ("W_hn_H", (96, 96), bf16), ("W_inn_att", (32, 96), bf16),
               ("W_r_HB", (96, 96), bf16), ("W_z_HB", (96, 96), bf16),
               ("W_inn_HB", (96, 96), bf16),
               ("b_r96", (96, 1), f32), ("b_z96", (96, 1), f32),
               ("b_inn96", (96, 1), f32), ("b_hn96", (96, 1), f32)]
    for gname in "IFGO":
        wspecs.append((f"L_{gname}_X", (96, 96), bf16))
        wspecs.append((f"L_{gname}_HL", (96, 96), bf16))
        wspecs.append((f"b_{gname}96", (96, 1), f32))
    wspecs += [("We1", (96, 48), bf16), ("be1", (48, 1), f32),
               ("We2", (48, 36), bf16), ("be2", (36, 1), f32),
               ("We3", (36, 6), bf16), ("be3", (6, 1), f32)]

    wnames = {}
    for nm, shp, dt in wspecs:
        wnames[nm] = nc.dram_tensor(nm, list(shp), dt, kind="ExternalInput")
    out_d = nc.dram_tensor("out", [6, NCORE], f32, kind="ExternalOutput")
    RA_QUAD = {0: 0, 1: 32, 2: 64, 3: 96}  # row-quadrant per RA matmul

    global WEIGHT_SPECS
    WEIGHT_SPECS = wspecs

    with tile.TileContext(nc) as tc:
        with ExitStack() as ctx:
            wpool = ctx.enter_context(tc.tile_pool(name="weights", bufs=1))
            wt = {}
            for nm, shp, dt in wspecs:
                # lhsT base partition must match the rhs row quadrant:
                # RA{n} runs at row quadrant 32n; W_inn_att at quadrant 96.
                if nm.startswith("RA"):
                    q = RA_QUAD[int(nm[2])]
                    t = wpool.tile([128, shp[1]], dt, tag=f"w_{nm}")
                    nc.sync.dma_start(t[q:q + 32, :], wnames[nm][:])
                    wt[nm] = t
                elif nm == "W_inn_att":
                    t = wpool.tile([128, shp[1]], dt, tag=f"w_{nm}")
                    nc.sync.dma_start(t[96:128, :], wnames[nm][:])
                    wt[nm] = t
                else:
                    t = wpool.tile([shp[0], shp[1]], dt, tag=f"w_{nm}")
                    nc.sync.dma_start(t[:], wnames[nm][:])
                    wt[nm] = t

            hpool = ctx.enter_context(tc.tile_pool(name="hg", bufs=3))
            arpool = ctx.enter_context(tc.tile_pool(name="ar", bufs=2))
            sbp = ctx.enter_context(tc.tile_pool(name="work", bufs=4))
            lstmp = ctx.enter_context(tc.tile_pool(name="lstm", bufs=3))
            psp = ctx.enter_context(tc.tile_pool(name="ps", bufs=2, space="PSUM"))

            def emit_tile(it):
                c0 = it * NB
                # ---- load groups ----
                HG = []
                for g in range(NGROUPS):
                    hg = hpool.tile([128, NB], bf16, tag=f"HG{g}")
                    sa = 2 * g
                    if g == 2:
                        # zero slot-b rows first (partition offsets must be 32-aligned;
                        # DMAs below restore the overlapping real rows)
                        nc.vector.memset(hg[32:64, :], 0.0)
                        nc.vector.memset(hg[64:96, :], 0.0)
                        nc.vector.memset(hg[96:128, :], 0.0)
                    nc.sync.dma_start(hg[0:48, :], feat_d[sa, :, c0:c0 + NB])
                    nc.sync.dma_start(hg[96:112, :], att_d[sa, :, c0:c0 + NB])
                    if g < 2:
                        sb = 2 * g + 1
                        nc.sync.dma_start(hg[48:96, :], feat_d[sb, :, c0:c0 + NB])
                        nc.sync.dma_start(hg[112:128, :], att_d[sb, :, c0:c0 + NB])
                    HG.append(hg)

                # att replicated into all 4 row-quadrants so the 4 RA matmuls can
                # run concurrently in distinct PE row groups (quadrant 3 = hg).
                AT4 = []
                for g in range(NGROUPS):
                    at4 = hpool.tile([96, NB], bf16, tag=f"AT4{g}")
                    if g == 2:
                        nc.vector.memset(at4[:], 0.0)
                    for q in range(3):
                        nc.sync.dma_start(at4[32 * q:32 * q + 16, :],
                                          att_d[2 * g, :, c0:c0 + NB])
                        if g < 2:
                            nc.sync.dma_start(at4[32 * q + 16:32 * q + 32, :],
                                              att_d[2 * g + 1, :, c0:c0 + NB])
                    AT4.append(at4)

                # ---- attention replication for all groups ----
                ARBG = {}
                for g in range(NGROUPS):
                    hg = HG[g]
                    ps_a = psp.tile([96, 2 * NB], f32, tag="ps_rz", name=f"psa_{it}_{g}")
                    ps_b = psp.tile([96, 2 * NB], f32, tag="ps_nh", name=f"psb_{it}_{g}")
                    for n in range(4):
                        ps = (ps_a, ps_b)[n // 2]
                        sl = ps[:, (n % 2) * NB:(n % 2 + 1) * NB]
                        q = RA_QUAD[n]
                        rhs = (hg[96:128, :] if n == 3
                               else AT4[g][q:q + 32, :])
                        nc.tensor.matmul(sl, wt[f"RA{n}"][q:q + 32, :], rhs,
                                         start=True, stop=True,
                                         tile_position=(q, 0))
                        ar = arpool.tile([96, NB], bf16, tag=f"ARB{g}_{n}")
                        if n % 2 == 0:
                            nc.vector.tensor_copy(ar[:], sl)
                        else:
                            nc.scalar.copy(ar[:], sl)
                        ARBG[(g, n)] = ar

                yield  # phase: loads + AR emitted
                # ---- 2 GRU passes, groups interleaved (3 independent chains) ----
                for pas in range(2):
                    for g in range(NGROUPS):
                        if g == 0 and pas == 1:
                            yield  # phase: pass 0 emitted
                        hg = HG[g]
                        ARB = [ARBG[(g, n)] for n in range(4)]
                        # Q_n = arb_n * h (both slots at once)
                        Q = []
                        for n in range(4):
                            q = sbp.tile([96, NB], bf16, tag=f"Q{n}")
                            if n >= 2:
                                nc.gpsimd.tensor_tensor(q[:], ARB[n][:], hg[0:96, :],
                                                        ALU.mult)
                            else:
                                nc.vector.tensor_mul(q[:], ARB[n][:], hg[0:96, :])
                            Q.append(q)
                        # hbar = sum_n SR_n @ Q_n
                        ps_hb = psp.tile([96, 2 * NB], f32, tag="ps_nh",
                                         name=f"pshb_{it}_{g}_{pas}")
                        for n in range(4):
                            nc.tensor.matmul(ps_hb[:, 0:NB], wt[f"SR{n}"][:], Q[n][:],
                                             start=(n == 0), stop=(n == 3))
                        hb = sbp.tile([96, NB], bf16, tag="HBs")
                        nc.vector.tensor_copy(hb[:], ps_hb[:, 0:NB])

                        # gates
                        ps_rz = psp.tile([96, 2 * NB], f32, tag="ps_rz",
                                         name=f"psrz_{it}_{g}_{pas}")
                        nc.tensor.matmul(ps_rz[:, 0:NB], wt["W_r_H"][:], hg[0:128, :],
                                         start=True, stop=False)
                        nc.tensor.matmul(ps_rz[:, 0:NB], wt["W_r_HB"][:], hb[:],
                                         start=False, stop=True)
                        nc.tensor.matmul(ps_rz[:, NB:2 * NB], wt["W_z_H"][:], hg[0:128, :],
                                         start=True, stop=False)
                        nc.tensor.matmul(ps_rz[:, NB:2 * NB], wt["W_z_HB"][:], hb[:],
                                         start=False, stop=True)
                        ps_nh = psp.tile([96, 2 * NB], f32, tag="ps_nh",
                                         name=f"psnh_{it}_{g}_{pas}")
                        nc.tensor.matmul(ps_nh[:, 0:NB], wt["W_inn_att"][96:128, :],
                                         hg[96:128, :], start=True, stop=False,
                                         tile_position=(96, 0))
                        nc.tensor.matmul(ps_nh[:, 0:NB], wt["W_inn_HB"][:], hb[:],
                                         start=False, stop=True)
                        nc.tensor.matmul(ps_nh[:, NB:2 * NB], wt["W_hn_H"][:],
                                         hg[0:96, :], start=True, stop=True)

                        srz = sbp.tile([96, 2 * NB], bf16, tag="SRZ")
                        nc.scalar.activation(srz[:, 0:NB], ps_rz[:, 0:NB], AF.Sigmoid,
                                             bias=wt["b_r96"][:, 0:1])
                        nc.scalar.activation(srz[:, NB:2 * NB], ps_rz[:, NB:2 * NB],
                                             AF.Sigmoid, bias=wt["b_z96"][:, 0:1])
                        t1 = sbp.tile([96, NB], f32, tag="t1")
                        nc.vector.scalar_tensor_tensor(t1[:], ps_nh[:, NB:2 * NB],
                                                       wt["b_hn96"][:, 0:1],
                                                       srz[:, 0:NB],
                                                       ALU.add, ALU.mult)
                        u = sbp.tile([96, NB], f32, tag="u")
                        nc.vector.scalar_tensor_tensor(u[:], ps_nh[:, 0:NB],
                                                       wt["b_inn96"][:, 0:1], t1[:],
                                                       ALU.add, ALU.add)
                        tn = sbp.tile([96, NB], bf16, tag="tn")
                        nc.scalar.activation(tn[:], u[:], AF.Tanh)
                        v = sbp.tile([96, NB], bf16, tag="v")
                        nc.vector.tensor_sub(v[:], hg[0:96, :], tn[:])
                        w2 = sbp.tile([96, NB], bf16, tag="w2")
                        nc.vector.tensor_mul(w2[:], srz[:, NB:2 * NB], v[:])
                        nc.vector.tensor_add(hg[0:96, :], tn[:], w2[:])

                yield  # phase: GRU emitted
                # ---- BiLSTM over S steps ----
                HL = lstmp.tile([96, NB], bf16, tag="HL", name=f"hl_{it}")
                C = lstmp.tile([96, NB], bf16, tag="C", name=f"c_{it}")
                nc.vector.memset(HL[:], 0.0)
                nc.vector.memset(C[:], 0.0)
                for t in range(S):
                    if t > 0:
                        yield  # phase: LSTM step t-1 emitted (ping-pong tiles)
                    sf_, sb_ = t, 4 - t
                    xl = lstmp.tile([96, NB], bf16, tag="XL")
                    nc.sync.dma_start(
                        xl[0:48, :], HG[sf_ // 2][(sf_ % 2) * 48:(sf_ % 2) * 48 + 48, :])
                    nc.sync.dma_start(
                        xl[48:96, :], HG[sb_ // 2][(sb_ % 2) * 48:(sb_ % 2) * 48 + 48, :])
                    ps_if = psp.tile([96, 2 * NB], f32, tag="ps_rz",
                                     name=f"psif_{it}_{t}")
                    ps_go = psp.tile([96, 2 * NB], f32, tag="ps_nh",
                                     name=f"psgo_{it}_{t}")
                    for gname, ps, half in (("I", ps_if, 0), ("F", ps_if, 1),
                                            ("G", ps_go, 0), ("O", ps_go, 1)):
                        sl = ps[:, half * NB:(half + 1) * NB]
                        nc.tensor.matmul(sl, wt[f"L_{gname}_X"][:], xl[:],
                                         start=True, stop=False)
                        nc.tensor.matmul(sl, wt[f"L_{gname}_HL"][:], HL[:],
                                         start=False, stop=True)
                    si = sbp.tile([96, NB], bf16, tag="si")
                    nc.scalar.activation(si[:], ps_if[:, 0:NB], AF.Sigmoid,
                                         bias=wt["b_I96"][:, 0:1])
                    sf2 = sbp.tile([96, NB], bf16, tag="sf2")
                    nc.scalar.activation(sf2[:], ps_if[:, NB:2 * NB], AF.Sigmoid,
                                         bias=wt["b_F96"][:, 0:1])
                    tg = sbp.tile([96, NB], bf16, tag="tg")
                    nc.scalar.activation(tg[:], ps_go[:, 0:NB], AF.Tanh,
                                         bias=wt["b_G96"][:, 0:1])
                    t1l = sbp.tile([96, NB], bf16, tag="t1l")
                    nc.vector.tensor_mul(t1l[:], si[:], tg[:])
                    t2l = sbp.tile([96, NB], bf16, tag="t2l")
                    nc.gpsimd.tensor_mul(t2l[:], sf2[:], C[:])
                    nc.vector.tensor_add(C[:], t1l[:], t2l[:])
                    tc2 = sbp.tile([96, NB], bf16, tag="tc2")
                    nc.scalar.activation(tc2[:], C[:], AF.Tanh)
                    so = sbp.tile([96, NB], bf16, tag="so")
                    nc.scalar.activation(so[:], ps_go[:, NB:2 * NB], AF.Sigmoid,
                                         bias=wt["b_O96"][:, 0:1])
                    nc.vector.tensor_mul(HL[:], so[:], tc2[:])

                yield  # phase: LSTM emitted
                # ---- MLP ----
                psE = psp.tile([96, 2 * NB], f32, tag="ps_rz", name=f"psE_{it}")
                nc.tensor.matmul(psE[0:48, 0:NB], wt["We1"][:], HL[:],
                                 start=True, stop=True)
                e1 = sbp.tile([48, NB], bf16, tag="e1")
                nc.scalar.activation(e1[:], psE[0:48, 0:NB], AF.Relu,
                                     bias=wt["be1"][:, 0:1])
                psE2 = psp.tile([96, 2 * NB], f32, tag="ps_nh", name=f"psE2_{it}")
                nc.tensor.matmul(psE2[0:36, 0:NB], wt["We2"][:], e1[:],
                                 start=True, stop=True)
                e2 = sbp.tile([36, NB], bf16, tag="e2")
                nc.scalar.activation(e2[:], psE2[0:36, 0:NB], AF.Relu,
                                     bias=wt["be2"][:, 0:1])
                nc.tensor.matmul(psE[0:6, NB:2 * NB], wt["We3"][:], e2[:],
                                 start=True, stop=True)
                o = sbp.tile([6, NB], f32, tag="o")
                nc.scalar.activation(o[:], psE[0:6, NB:2 * NB], AF.Identity,
                                     bias=wt["be3"][:, 0:1])
                nc.sync.dma_start(out_d[:, c0:c0 + NB], o[:])

            # Interleave tile pairs at phase granularity so one tile's matmuls
            # fill the other's activation/DVE stalls (esp. the LSTM recurrence).
            for k in range(0, NTILES, 2):
                gens = [emit_tile(k), emit_tile(k + 1)]
                done = [False, False]
                while not all(done):
                    for i, gg in enumerate(gens):
                        if not done[i]:
                            try:
                                next(gg)
                            except StopIteration:
                                done[i] = True

    split_excess_waits(nc)
    return nc


_NC_CACHE = None
TRACE = False
LAST_EXEC_NS = None


def kernel(nodes_feature, pos, attmat, **w):
    global _NC_CACHE, LAST_EXEC_NS
    from concourse.bass_utils import run_bass_kernel_spmd
    import concourse.mybir as mybir

    feat, att = prep_inputs(nodes_feature, pos, attmat)
    wts = build_weights(**w)

    if _NC_CACHE is None:
        _NC_CACHE = build_nc()
    nc = _NC_CACHE

    in_maps = []
    for c in range(NCORES):
        m = {"feat": np.ascontiguousarray(feat[:, :, c * NCORE:(c + 1) * NCORE]),
             "att": np.ascontiguousarray(att[:, :, c * NCORE:(c + 1) * NCORE])}
        for nm, shp, dt in WEIGHT_SPECS:
            m[nm] = wts[nm].astype(BF) if dt == mybir.dt.bfloat16 else wts[nm].astype(np.float32)
        in_maps.append(m)

    res = run_bass_kernel_spmd(nc, in_maps, core_ids=list(range(NCORES)),
                               trace=TRACE)
    LAST_EXEC_NS = res.exec_time_ns
    outs = [res.results[c]["out"] for c in range(NCORES)]     # [6, NCORE] each
    full = np.concatenate(outs, axis=1)                        # [6, N]
    return np.ascontiguousarray(full.T).astype(np.float32)     # [N, 6]


# revision 41
# speedup vs baseline: 1.6011x; 1.0550x over previous
"""Trainium2 Bass kernel for nn_Atomic_node_only_lstm (GNN message passing + BiLSTM + MLP).

v2: s-paired GRU with msg-linear folded into the gate matmuls.

Data-parallel over 8 NeuronCores (batch N=65536 -> 8192/core), NB=512 cols/tile.

Device layout (per batch tile of NB columns), seq steps grouped in pairs
(groups: (s0,s1), (s2,s3), (s4,zero)):
  H_g  [128, NB] bf16: rows 0:48 h_a, 48:96 h_b, 96:112 att_a, 112:128 att_b
       (h rows indexed k*12+d, k = node; att rows indexed n*4+w)
  arb_n [96, NB]: att[n,w] replicated over d, slot a rows 0:48, slot b 48:96
  Q_n  [96, NB] = arb_n * H_g[0:96]   (q_n[w*12+d] = att[n,w]*h[w,d], both slots)
  hbar [96, NB]: rows j*48+n*12+d = sum_w q over w   (slot j, node n)
  GRU gates from K=128 contraction of H_g (h part + att part folds s_n*(W_ih@msg_b))
  plus K=96 contraction of hbar (folds (W_ih@msg_w)).
  PSUM super-tiles [96, 1024] span 2 banks: rz = [r | z], nh = [inn | hn].

LSTM: XL [96, NB] = [x_fw | x_bw], HL [96, NB] = [h_fw | h_bw], C [96, NB].
  Per gate-type bank halves in free dim: B_if = [i | f], B_go = [g | o].
  x[j] = h[(k,d)], j = d*4+k (handled in lhsT construction).
MLP reads HL directly (en1_w cols 0:48 fw, 48:96 bw).
"""
import numpy as np
import ml_dtypes
from contextlib import ExitStack

N, S, K, D, H = 65536, 5, 4, 12, 48
NCORES = 8
NCORE = N // NCORES          # 8192 batch elements per core
NB = 512                     # batch columns per tile
NTILES = NCORE // NB
NGROUPS = 3                  # s-pairs: (0,1), (2,3), (4,-)

BF = ml_dtypes.bfloat16


# ----------------------------------------------------------------------------
# host-side weight construction (numpy, all tiny)
# ----------------------------------------------------------------------------
def build_weights(msg_w, msg_b, gru_w_ih, gru_w_hh, gru_b_ih, gru_b_hh,
                  lstm_w_ih_fw, lstm_w_hh_fw, lstm_b_ih_fw, lstm_b_hh_fw,
                  lstm_w_ih_bw, lstm_w_hh_bw, lstm_b_ih_bw, lstm_b_hh_bw,
                  en1_w, en1_b, en2_w, en2_b, en3_w, en3_b):
    out = {}
    # composed message->gate matrices [12 out-gate-d, 12 in-feat]
    A_r = gru_w_ih[0:12] @ msg_w      # gi_r = A_r @ hbar + s*(w_ih_r@msg_b)
    A_z = gru_w_ih[12:24] @ msg_w
    A_n = gru_w_ih[24:36] @ msg_w
    bi_r = gru_w_ih[0:12] @ msg_b     # [12]
    bi_z = gru_w_ih[12:24] @ msg_b
    bi_n = gru_w_ih[24:36] @ msg_b

    # RA_n lhsT [32, 96] (placed at partitions 96:128 on device; rhs = H_g[96:128]).
    # rows: 0:16 att_a (n*4+w), 16:32 att_b. cols: j*48 + w*12 + d.
    for n in range(4):
        R = np.zeros((32, 96), np.float32)
        for j in range(2):
            for w in range(4):
                for d in range(12):
                    R[j * 16 + n * 4 + w, j * 48 + w * 12 + d] = 1.0
        out[f"RA{n}"] = R

    # SR_n lhsT [96, 96]: contract Q_n -> hbar contribution of node n.
    # Q_n rows j*48 + w*12 + d ; hbar cols j*48 + n*12 + d.
    for n in range(4):
        Sm = np.zeros((96, 96), np.float32)
        for j in range(2):
            for w in range(4):
                for d in range(12):
                    Sm[j * 48 + w * 12 + d, j * 48 + n * 12 + d] = 1.0
        out[f"SR{n}"] = Sm

    # GRU gate lhsTs.
    # H-part [128, 96]: rhs = H_g (h rows j*48+k*12+dp, att rows 96+j*16+k*4+w)
    # out cols j*48 + k*12 + d.
    def gate_H(whh_blk, bi_blk):
        Wm = np.zeros((128, 96), np.float32)
        for j in range(2):
            for k in range(4):
                for d in range(12):
                    c = j * 48 + k * 12 + d
                    if whh_blk is not None:
                        for dp in range(12):
                            Wm[j * 48 + k * 12 + dp, c] = whh_blk[d, dp]
                    if bi_blk is not None:
                        for w in range(4):
                            Wm[96 + j * 16 + k * 4 + w, c] = bi_blk[d]
        return Wm

    def gate_HB(A_blk):
        Wm = np.zeros((96, 96), np.float32)
        for j in range(2):
            for k in range(4):
                for d in range(12):
                    c = j * 48 + k * 12 + d
                    for dp in range(12):
                        Wm[j * 48 + k * 12 + dp, c] = A_blk[d, dp]
        return Wm

    out["W_r_H"] = gate_H(gru_w_hh[0:12], bi_r)
    out["W_z_H"] = gate_H(gru_w_hh[12:24], bi_z)
    out["W_hn_H"] = gate_H(gru_w_hh[24:36], None)[0:96]   # h rows only, K=96
    out["W_inn_att"] = gate_H(None, bi_n)[96:128]         # att rows only, K=32
    out["W_r_HB"] = gate_HB(A_r)
    out["W_z_HB"] = gate_HB(A_z)
    out["W_inn_HB"] = gate_HB(A_n)

    out["b_r96"] = np.tile(gru_b_ih[0:12] + gru_b_hh[0:12], 8).reshape(96, 1).astype(np.float32)
    out["b_z96"] = np.tile(gru_b_ih[12:24] + gru_b_hh[12:24], 8).reshape(96, 1).astype(np.float32)
    out["b_inn96"] = np.tile(gru_b_ih[24:36], 8).reshape(96, 1).astype(np.float32)
    out["b_hn96"] = np.tile(gru_b_hh[24:36], 8).reshape(96, 1).astype(np.float32)
    # rz super-bias [96,1] applies to both halves? No - bias is per partition, free
    # dim halves share it. r and z biases differ -> separate ACT calls? No:
    # sigma(rz super-tile) is ONE act over [96, 1024]; bias per-partition only.
    # So we must fold r/z biases into the matmuls instead: add via att-part?
    # Simpler: bias rows are per-partition; r occupies cols 0:512 and z 512:1024 of
    # the SAME partitions -> per-partition bias cannot differ. Fold biases into
    # lhsT via the constant-1 trick: use att rows? att sums vary. Instead keep
    # two ACT calls when biases differ. For rz we instead ADD the bias inside the
    # H-part lhsT using a constant row... no constant row exists in H_g.
    # Resolution: biases b_r/b_z are added via activation bias -> need separate
    # sigma(r), sigma(z). To keep ONE act per super-tile we instead pre-add the
    # bias by augmenting att columns: s_n terms already use att rows; a constant
    # offset cannot come from data. So: two ACTs for rz after all (see kernel).

    # LSTM lhsTs.
    # XL-part [96, 96]: rhs = XL = [x_fw | x_bw]; rows j*48 + k*12 + d (x of dir j)
    # cols j*48 + m. x[jj] = h[(k,d)], jj = d*4+k -> weight w_ih[g*48+m, d*4+k].
    # HL-part [96, 96]: rhs = HL = [h_fw | h_bw]; rows j*48 + jj; cols j*48 + m.
    gates = {"I": 0, "F": 1, "G": 2, "O": 3}
    wih = {"fw": lstm_w_ih_fw, "bw": lstm_w_ih_bw}
    whh = {"fw": lstm_w_hh_fw, "bw": lstm_w_hh_bw}
    bih = {"fw": lstm_b_ih_fw, "bw": lstm_b_ih_bw}
    bhh = {"fw": lstm_b_hh_fw, "bw": lstm_b_hh_bw}
    for gname, g in gates.items():
        X = np.zeros((96, 96), np.float32)
        Hm = np.zeros((96, 96), np.float32)
        b = np.zeros((96, 1), np.float32)
        for j, dirn in enumerate(("fw", "bw")):
            wg = wih[dirn][g * 48:(g + 1) * 48, :]   # [48 out m, 48 xin jj]
            hg = whh[dirn][g * 48:(g + 1) * 48, :]
            for k in range(4):
                for d in range(12):
                    X[j * 48 + k * 12 + d, j * 48:(j + 1) * 48] = wg[:, d * 4 + k]
            Hm[j * 48:(j + 1) * 48, j * 48:(j + 1) * 48] = hg.T
            b[j * 48:(j + 1) * 48, 0] = (bih[dirn][g * 48:(g + 1) * 48]
                                         + bhh[dirn][g * 48:(g + 1) * 48])
        out[f"L_{gname}_X"] = X
        out[f"L_{gname}_HL"] = Hm
        out[f"b_{gname}96"] = b

    # MLP: HL = [h_fw(0:48) | h_bw(48:96)]
    We1 = np.zeros((96, 48), np.float32)
    We1[0:48, :] = en1_w[:, 0:48].T
    We1[48:96, :] = en1_w[:, 48:96].T
    out["We1"] = We1
    out["be1"] = en1_b.reshape(48, 1).astype(np.float32)
    out["We2"] = en2_w.T.copy()                 # [48, 36]
    out["be2"] = en2_b.reshape(36, 1).astype(np.float32)
    out["We3"] = en3_w.T.copy()                 # [36, 6]
    out["be3"] = en3_b.reshape(6, 1).astype(np.float32)
    return out


def prep_inputs(nodes_feature, pos, attmat):
    """Full-batch host layout: feat [S, 48, N] bf16 (k-major (k,d)), att [S, 16, N] bf16."""
    nf = np.concatenate([nodes_feature, pos], axis=-1)       # [N,S,K,12]
    feat = np.ascontiguousarray(nf.transpose(1, 2, 3, 0)).reshape(S, 48, N)
    att = np.ascontiguousarray(attmat.transpose(1, 2, 3, 0)).reshape(S, 16, N)
    return feat.astype(BF), att.astype(BF)


# ----------------------------------------------------------------------------
# device kernel builder
# ----------------------------------------------------------------------------
def split_excess_waits(nc, max_waits=1):
    import concourse.mybir as mybir
    cnt = 0
    for f in nc.m.functions:
        for bb in f.blocks:
            insts = bb.instructions
            new = []
            changed = False
            for inst in insts:
                si = inst.sync_info
                waits = list(si.on_wait) if si and si.on_wait else []
                if len(waits) > max_waits:
                    changed = True
                    k = 0
                    while len(waits) - k > max_waits:
                        chunk = waits[k:k + max_waits]
                        k += max_waits
                        cnt += 1
                        nop = mybir.InstNoOp(name=f"waitsplit-{cnt}", ins=[], outs=[])
                        nop.engine = inst.engine
                        nop.sync_info = mybir.SyncInfo(on_wait=chunk, on_update=[])
                        new.append(nop)
                    inst.sync_info = mybir.SyncInfo(
                        on_wait=waits[k:],
                        on_update=list(si.on_update) if si.on_update else [])
                new.append(inst)
            if changed:
                bb.instructions = new
    return cnt


WEIGHT_SPECS = None  # filled in build_nc


def build_nc():
    import concourse.bass as bass
    import concourse.tile as tile
    from concourse import mybir

    f32 = mybir.dt.float32
    bf16 = mybir.dt.bfloat16
    AF = mybir.ActivationFunctionType
    ALU = mybir.AluOpType

    nc = bass.Bass("TRN2")

    feat_d = nc.dram_tensor("feat", [S, 48, NCORE], bf16, kind="ExternalInput")
    att_d = nc.dram_tensor("att", [S, 16, NCORE], bf16, kind="ExternalInput")

    wspecs = []
    for n in range(4):
        wspecs.append((f"RA{n}", (32, 96), bf16))
        wspecs.append((f"SR{n}", (96, 96), bf16))
    wspecs += [("W_r_H", (128, 96), bf16), ("W_z_H", (128, 96), bf16),
               ("W_hn_H", (96, 96), bf16), ("W_inn_att", (32, 96), bf16),
               ("W_r_HB", (96, 96), bf16), ("W_z_HB", (96, 96), bf16),
               ("W_inn_HB", (96, 96), bf16),
               ("b_r96", (96, 1), f32), ("b_z96", (96, 1), f32),
               ("b_inn96", (96, 1), f32), ("b_hn96", (96, 1), f32)]
    for gname in "IFGO":
        wspecs.append((f"L_{gname}_X", (96, 96), bf16))
        wspecs.append((f"L_{gname}_HL", (96, 96), bf16))
        wspecs.append((f"b_{gname}96", (96, 1), f32))
    wspecs += [("We1", (96, 48), bf16), ("be1", (48, 1), f32),
               ("We2", (48, 36), bf16), ("be2", (36, 1), f32),
               ("We3", (36, 6), bf16), ("be3", (6, 1), f32)]

    wnames = {}
    for nm, shp, dt in wspecs:
        wnames[nm] = nc.dram_tensor(nm, list(shp), dt, kind="ExternalInput")
    out_d = nc.dram_tensor("out", [6, NCORE], f32, kind="ExternalOutput")
    RA_QUAD = {0: 0, 1: 32, 2: 64, 3: 96}  # row-quadrant per RA matmul

    global WEIGHT_SPECS
    WEIGHT_SPECS = wspecs

    with tile.TileContext(nc) as tc:
        with ExitStack() as ctx:
            wpool = ctx.enter_context(tc.tile_pool(name="weights", bufs=1))
            wt = {}
            for nm, shp, dt in wspecs:
                # lhsT base partition must match the rhs row quadrant:
                # RA{n} runs at row quadrant 32n; W_inn_att at quadrant 96.
                if nm.startswith("RA"):
                    q = RA_QUAD[int(nm[2])]
                    t = wpool.tile([128, shp[1]], dt, tag=f"w_{nm}")
                    nc.sync.dma_start(t[q:q + 32, :], wnames[nm][:])
                    wt[nm] = t
                elif nm == "W_inn_att":
                    t = wpool.tile([128, shp[1]], dt, tag=f"w_{nm}")
                    nc.sync.dma_start(t[96:128, :], wnames[nm][:])
                    wt[nm] = t
                else:
                    t = wpool.tile([shp[0], shp[1]], dt, tag=f"w_{nm}")
                    nc.sync.dma_start(t[:], wnames[nm][:])
                    wt[nm] = t

            hpool = ctx.enter_context(tc.tile_pool(name="hg", bufs=3))
            arpool = ctx.enter_context(tc.tile_pool(name="ar", bufs=2))
            sbp = ctx.enter_context(tc.tile_pool(name="work", bufs=4))
            lstmp = ctx.enter_context(tc.tile_pool(name="lstm", bufs=3))
            psp = ctx.enter_context(tc.tile_pool(name="ps", bufs=2, space="PSUM"))

            def emit_tile(it):
                c0 = it * NB
                # ---- load groups ----
                HG = []
                for g in range(NGROUPS):
                    hg = hpool.tile([128, NB], bf16, tag=f"HG{g}")
                    sa = 2 * g
                    if g == 2:
                        # zero slot-b rows first (partition offsets must be 32-aligned;
                        # DMAs below restore the overlapping real rows)
                        nc.vector.memset(hg[32:64, :], 0.0)
                        nc.vector.memset(hg[64:96, :], 0.0)
                        nc.vector.memset(hg[96:128, :], 0.0)
                    nc.sync.dma_start(hg[0:48, :], feat_d[sa, :, c0:c0 + NB])
                    nc.sync.dma_start(hg[96:112, :], att_d[sa, :, c0:c0 + NB])
                    if g < 2:
                        sb = 2 * g + 1
                        nc.sync.dma_start(hg[48:96, :], feat_d[sb, :, c0:c0 + NB])
                        nc.sync.dma_start(hg[112:128, :], att_d[sb, :, c0:c0 + NB])
                    HG.append(hg)

                # att replicated into all 4 row-quadrants so the 4 RA matmuls can
                # run concurrently in distinct PE row groups (quadrant 3 = hg).
                AT4 = []
                for g in range(NGROUPS):
                    at4 = hpool.tile([96, NB], bf16, tag=f"AT4{g}")
                    if g == 2:
                        nc.vector.memset(at4[:], 0.0)
                    for q in range(3):
                        nc.sync.dma_start(at4[32 * q:32 * q + 16, :],
                                          att_d[2 * g, :, c0:c0 + NB])
                        if g < 2:
                            nc.sync.dma_start(at4[32 * q + 16:32 * q + 32, :],
                                              att_d[2 * g + 1, :, c0:c0 + NB])
                    AT4.append(at4)

                # ---- attention replication for all groups ----
                ARBG = {}
                for g in range(NGROUPS):
                    hg = HG[g]
                    ps_a = psp.tile([96, 2 * NB], f32, tag="ps_rz", name=f"psa_{it}_{g}")
                    ps_b = psp.tile([96, 2 * NB], f32, tag="ps_nh", name=f"psb_{it}_{g}")
                    for n in range(4):
                        ps = (ps_a, ps_b)[n // 2]
                        sl = ps[:, (n % 2) * NB:(n % 2 + 1) * NB]
                        q = RA_QUAD[n]
                        rhs = (hg[96:128, :] if n == 3
                               else AT4[g][q:q + 32, :])
                        nc.tensor.matmul(sl, wt[f"RA{n}"][q:q + 32, :], rhs,
                                         start=True, stop=True,
                                         tile_position=(q, 0))
                        ar = arpool.tile([96, NB], bf16, tag=f"ARB{g}_{n}")
                        if n % 2 == 0:
                            nc.vector.tensor_copy(ar[:], sl)
                        else:
                            nc.scalar.copy(ar[:], sl)
                        ARBG[(g, n)] = ar

                yield  # phase: loads + AR emitted
                # ---- 2 GRU passes, groups interleaved (3 independent chains) ----
                for pas in range(2):
                    for g in range(NGROUPS):
                        if g == 0 and pas == 1:
                            yield  # phase: pass 0 emitted
                        hg = HG[g]
                        ARB = [ARBG[(g, n)] for n in range(4)]
                        # Q_n = arb_n * h (both slots at once)
                        Q = []
                        for n in range(4):
                            q = sbp.tile([96, NB], bf16, tag=f"Q{n}")
                            if n >= 2:
                                nc.gpsimd.tensor_tensor(q[:], ARB[n][:], hg[0:96, :],
                                                        ALU.mult)
                            else:
                                nc.vector.tensor_mul(q[:], ARB[n][:], hg[0:96, :])
                            Q.append(q)
                        # hbar = sum_n SR_n @ Q_n
                        ps_hb = psp.tile([96, 2 * NB], f32, tag="ps_nh",
                                         name=f"pshb_{it}_{g}_{pas}")
                        for n in range(4):
                            nc.tensor.matmul(ps_hb[:, 0:NB], wt[f"SR{n}"][:], Q[n][:],
                                             start=(n == 0), stop=(n == 3))
                        hb = sbp.tile([96, NB], bf16, tag="HBs")
                        nc.vector.tensor_copy(hb[:], ps_hb[:, 0:NB])

                        # gates
                        ps_rz = psp.tile([96, 2 * NB], f32, tag="ps_rz",
                                         name=f"psrz_{it}_{g}_{pas}")
                        nc.tensor.matmul(ps_rz[:, 0:NB], wt["W_r_H"][:], hg[0:128, :],
                                         start=True, stop=False)
                        nc.tensor.matmul(ps_rz[:, 0:NB], wt["W_r_HB"][:], hb[:],
                                         start=False, stop=True)
                        nc.tensor.matmul(ps_rz[:, NB:2 * NB], wt["W_z_H"][:], hg[0:128, :],
                                         start=True, stop=False)
                        nc.tensor.matmul(ps_rz[:, NB:2 * NB], wt["W_z_HB"][:], hb[:],
                                         start=False, stop=True)
                        ps_nh = psp.tile([96, 2 * NB], f32, tag="ps_nh",
                                         name=f"psnh_{it}_{g}_{pas}")
                        nc.tensor.matmul(ps_nh[:, 0:NB], wt["W_inn_att"][96:128, :],
                                         hg[96:128, :], start=True, stop=False,
                                         tile_position=(96, 0))
                        nc.tensor.matmul(ps_nh[:, 0:NB], wt["W_inn_HB"][:], hb[:],
                                         start=False, stop=True)
                        nc.tensor.matmul(ps_nh[:, NB:2 * NB], wt["W_hn_H"][:],
                                         hg[0:96, :], start=True, stop=True)

                        srz = sbp.tile([96, 2 * NB], bf16, tag="SRZ")
                        nc.scalar.activation(srz[:, 0:NB], ps_rz[:, 0:NB], AF.Sigmoid,
                                             bias=wt["b_r96"][:, 0:1])
                        nc.scalar.activation(srz[:, NB:2 * NB], ps_rz[:, NB:2 * NB],
                                             AF.Sigmoid, bias=wt["b_z96"][:, 0:1])
                        t1 = sbp.tile([96, NB], f32, tag="t1")
                        nc.vector.scalar_tensor_tensor(t1[:], ps_nh[:, NB:2 * NB],
                                                       wt["b_hn96"][:, 0:1],
                                                       srz[:, 0:NB],
                                                       ALU.add, ALU.mult)
                        u = sbp.tile([96, NB], f32, tag="u")
                        nc.vector.scalar_tensor_tensor(u[:], ps_nh[:, 0:NB],
                                                       wt["b_inn96"][:, 0:1], t1[:],
                                                       ALU.add, ALU.add)
                        tn = sbp.tile([96, NB], bf16, tag="tn")
                        nc.scalar.activation(tn[:], u[:], AF.Tanh)
                        v = sbp.tile([96, NB], bf16, tag="v")
                        nc.vector.tensor_sub(v[:], hg[0:96, :], tn[:])
                        w2 = sbp.tile([96, NB], bf16, tag="w2")
                        nc.vector.tensor_mul(w2[:], srz[:, NB:2 * NB], v[:])
                        nc.vector.tensor_add(hg[0:96, :], tn[:], w2[:])

                yield  # phase: GRU emitted
                # ---- BiLSTM over S steps ----
                HL = lstmp.tile([96, NB], bf16, tag="HL", name=f"hl_{it}")
                C = lstmp.tile([96, NB], bf16, tag="C", name=f"c_{it}")
                # no memsets needed: t=0 skips all reads of HL and C
                for t in range(S):
                    if t > 0:
                        yield  # phase: LSTM step t-1 emitted (ping-pong tiles)
                    sf_, sb_ = t, 4 - t
                    xl = lstmp.tile([96, NB], bf16, tag="XL")
                    nc.sync.dma_start(
                        xl[0:48, :], HG[sf_ // 2][(sf_ % 2) * 48:(sf_ % 2) * 48 + 48, :])
                    nc.sync.dma_start(
                        xl[48:96, :], HG[sb_ // 2][(sb_ % 2) * 48:(sb_ % 2) * 48 + 48, :])
                    ps_if = psp.tile([96, 2 * NB], f32, tag="ps_rz",
                                     name=f"psif_{it}_{t}")
                    ps_go = psp.tile([96, 2 * NB], f32, tag="ps_nh",
                                     name=f"psgo_{it}_{t}")
                    for gname, ps, half in (("I", ps_if, 0), ("F", ps_if, 1),
                                            ("G", ps_go, 0), ("O", ps_go, 1)):
                        if t == 0 and gname == "F":
                            continue  # f-gate unused at t=0 (c0 = 0)
                        sl = ps[:, half * NB:(half + 1) * NB]
                        # t=0: h is zero, so the HL-part matmul is skipped
                        nc.tensor.matmul(sl, wt[f"L_{gname}_X"][:], xl[:],
                                         start=True, stop=(t == 0))
                        if t > 0:
                            nc.tensor.matmul(sl, wt[f"L_{gname}_HL"][:], HL[:],
                                             start=False, stop=True)
                    si = sbp.tile([96, NB], bf16, tag="si")
                    nc.scalar.activation(si[:], ps_if[:, 0:NB], AF.Sigmoid,
                                         bias=wt["b_I96"][:, 0:1])
                    tg = sbp.tile([96, NB], bf16, tag="tg")
                    nc.scalar.activation(tg[:], ps_go[:, 0:NB], AF.Tanh,
                                         bias=wt["b_G96"][:, 0:1])
                    t1l = sbp.tile([96, NB], bf16, tag="t1l")
                    nc.vector.tensor_mul(t1l[:], si[:], tg[:])
                    if t == 0:
                        # c = i*g (f-gate term is zero against c0=0)
                        nc.vector.tensor_copy(C[:], t1l[:])
                    else:
                        sf2 = sbp.tile([96, NB], bf16, tag="sf2")
                        nc.scalar.activation(sf2[:], ps_if[:, NB:2 * NB], AF.Sigmoid,
                                             bias=wt["b_F96"][:, 0:1])
                        t2l = sbp.tile([96, NB], bf16, tag="t2l")
                        nc.gpsimd.tensor_mul(t2l[:], sf2[:], C[:])
                        nc.vector.tensor_add(C[:], t1l[:], t2l[:])
                    tc2 = sbp.tile([96, NB], bf16, tag="tc2")
                    nc.scalar.activation(tc2[:], C[:], AF.Tanh)
                    so = sbp.tile([96, NB], bf16, tag="so")
                    nc.scalar.activation(so[:], ps_go[:, NB:2 * NB], AF.Sigmoid,
                                         bias=wt["b_O96"][:, 0:1])
                    nc.vector.tensor_mul(HL[:], so[:], tc2[:])

                yield  # phase: LSTM emitted
                # ---- MLP ----
                psE = psp.tile([96, 2 * NB], f32, tag="ps_rz", name=f"psE_{it}")
                nc.tensor.matmul(psE[0:48, 0:NB], wt["We1"][:], HL[:],
                                 start=True, stop=True)
                e1 = sbp.tile([48, NB], bf16, tag="e1")
                nc.scalar.activation(e1[:], psE[0:48, 0:NB], AF.Relu,
                                     bias=wt["be1"][:, 0:1])
                psE2 = psp.tile([96, 2 * NB], f32, tag="ps_nh", name=f"psE2_{it}")
                nc.tensor.matmul(psE2[0:36, 0:NB], wt["We2"][:], e1[:],
                                 start=True, stop=True)
                e2 = sbp.tile([36, NB], bf16, tag="e2")
                nc.scalar.activation(e2[:], psE2[0:36, 0:NB], AF.Relu,
                                     bias=wt["be2"][:, 0:1])
                nc.tensor.matmul(psE[0:6, NB:2 * NB], wt["We3"][:], e2[:],
                                 start=True, stop=True)
                o = sbp.tile([6, NB], f32, tag="o")
                nc.scalar.activation(o[:], psE[0:6, NB:2 * NB], AF.Identity,
                                     bias=wt["be3"][:, 0:1])
                nc.sync.dma_start(out_d[:, c0:c0 + NB], o[:])

            # Interleave tile pairs at phase granularity so one tile's matmuls
            # fill the other's activation/DVE stalls (esp. the LSTM recurrence).
            for k in range(0, NTILES, 2):
                gens = [emit_tile(k), emit_tile(k + 1)]
                done = [False, False]
                while not all(done):
                    for i, gg in enumerate(gens):
                        if not done[i]:
                            try:
                                next(gg)
                            except StopIteration:
                                done[i] = True

    split_excess_waits(nc)
    return nc


_NC_CACHE = None
TRACE = False
LAST_EXEC_NS = None


def kernel(nodes_feature, pos, attmat, **w):
    global _NC_CACHE, LAST_EXEC_NS
    from concourse.bass_utils import run_bass_kernel_spmd
    import concourse.mybir as mybir

    feat, att = prep_inputs(nodes_feature, pos, attmat)
    wts = build_weights(**w)

    if _NC_CACHE is None:
        _NC_CACHE = build_nc()
    nc = _NC_CACHE

    in_maps = []
    for c in range(NCORES):
        m = {"feat": np.ascontiguousarray(feat[:, :, c * NCORE:(c + 1) * NCORE]),
             "att": np.ascontiguousarray(att[:, :, c * NCORE:(c + 1) * NCORE])}
        for nm, shp, dt in WEIGHT_SPECS:
            m[nm] = wts[nm].astype(BF) if dt == mybir.dt.bfloat16 else wts[nm].astype(np.float32)
        in_maps.append(m)

    res = run_bass_kernel_spmd(nc, in_maps, core_ids=list(range(NCORES)),
                               trace=TRACE)
    LAST_EXEC_NS = res.exec_time_ns
    outs = [res.results[c]["out"] for c in range(NCORES)]     # [6, NCORE] each
    full = np.concatenate(outs, axis=1)                        # [6, N]
    return np.ascontiguousarray(full.T).astype(np.float32)     # [N, 6]
